# revision 9
# baseline (speedup 1.0000x reference)
"""Trainium2 Bass kernel for nn_CCAR_11579231830663 (dense_transformer).

Data-parallel over batch: 16 samples -> 8 NeuronCores x 2 samples. The global
z-score mean/std of x_g and g are the only cross-core terms; each core
all-reduces 4 scalar sums (sum/sumsq of x_g and g) on-device.

Per sample (C=512, W=1024):
  g   = sin(IN(conv3(x, rw1)+rb1)); g = sin(IN(conv3(g, rw2)+rb2))
  x_g = x + g
  qe  = zscore(x_g)^T . pq,  pq = qw@x_g + qb     (z-score over ALL of x_g)
  ke  = zscore(g)^T  . pk,  pk = kw@g + kb
  energy = qe @ ke ; att = softmax(energy); out = (vw@g+vb) @ att^T
The z-score is folded out: center x_g/g by the global means, compute raw
bilinear forms M1 = xc^T@pq (lhsT for energy), M2 = gc^T@pk (rhs), and fold
alpha = 1/(s_xg*s_g) into the exp: softmax(a*E) = exp(a*(E-rowmax))/sum, a>0.
"""
import sys
sys.path.insert(0, '/opt/trn_rl_repo')

import numpy as np
from contextlib import ExitStack

import concourse.bass as bass
import concourse.tile as tile
from concourse import mybir
from concourse.masks import make_identity
from concourse.bass_utils import run_bass_kernel_spmd

F32 = mybir.dt.float32
AF = mybir.ActivationFunctionType
ALU = mybir.AluOpType
AX = mybir.AxisListType

N_CORES = 8
B, C, W = 16, 512, 1024
SPC = B // N_CORES      # samples per core
CT = C // 128           # channel tiles
KT = W // 128           # width 128-tiles
EPS = 1e-5
NTOT = float(B * C * W)

MAGIC = 12582912.0       # 1.5*2^23 fp32 round-to-nearest-int magic
TWOPI = float(2 * np.pi)
INV2PI = float(1.0 / (2 * np.pi))

# ---------------------------------------------------------------------------
# walrus workaround: this container's walrus accepts only a limited number of
# sync waits per instruction; Tile can aggregate more (e.g. the tail drain).
# Split excess waits onto same-engine NOPs placed just before the instruction.
_uid = [0]


def _split_multiwait(nc, limit=1):
    for f in nc.m.functions:
        for bb in f.blocks:
            insts = list(bb.instructions)
            out = []
            changed = False
            for inst in insts:
                si = inst.sync_info
                waits = list(si.on_wait) if si is not None and si.on_wait else []
                if len(waits) > limit:
                    changed = True
                    excess, keep = waits[:-limit], waits[-limit:]
                    si.on_wait = keep
                    inst.sync_info = si
                    for i in range(0, len(excess), limit):
                        chunk = excess[i:i + limit]
                        _uid[0] += 1
                        nop = mybir.InstNoOp(
                            name=f"I-waitsplit-{_uid[0]}", ins=[], outs=[])
                        nop.engine = inst.engine
                        nop.sync_info = mybir.SyncInfo(
                            on_wait=chunk, on_update=[])
                        out.append(nop)
                out.append(inst)
            if changed:
                bb.instructions = out


# ---------------------------------------------------------------------------
def _emit(nc, tc, ctx, dram):
    V = nc.vector
    S = nc.scalar
    T = nc.tensor

    # ---------------- outer pools (small / long-lived) --------------------
    singles = ctx.enter_context(tc.tile_pool(name="singles", bufs=1))
    spool = ctx.enter_context(tc.tile_pool(name="spool", bufs=1))
    nrm = ctx.enter_context(tc.tile_pool(name="nrm", bufs=4))
    outbuf = ctx.enter_context(tc.tile_pool(name="outbuf", bufs=2))
    qkv_w = ctx.enter_context(tc.tile_pool(name="qkv_w", bufs=1))
    big = ctx.enter_context(tc.tile_pool(name="big", bufs=1))
    mm_psum = ctx.enter_context(
        tc.tile_pool(name="mm_psum", bufs=3, space="PSUM"))
    sm_psum = ctx.enter_context(
        tc.tile_pool(name="sm_psum", bufs=2, space="PSUM"))

    ident = singles.tile([128, 128], F32, name="ident")
    make_identity(nc, ident[:])
    ones1 = singles.tile([1, 128], F32, name="ones1")
    V.memset(ones1[:], 1.0)
    ones128 = singles.tile([128, 1], F32, name="ones128")
    V.memset(ones128[:], 1.0)

    def load_bias_cols(name):
        t = singles.tile([128, CT], F32, name=f"{name}_cols")
        src = dram[name].ap().rearrange("(t p) -> p t", p=128)
        nc.sync.dma_start(out=t[:], in_=src)
        return t

    rb1b = load_bias_cols("rb1")
    rb2b = load_bias_cols("rb2")
    qbb = load_bias_cols("qb")
    kbb = load_bias_cols("kb")
    vb_row = singles.tile([1, C], F32, name="vb_row")
    nc.sync.dma_start(out=vb_row[:], in_=dram["vb"].ap()[None, :])

    # columns: 0 sum_xg, 1 sumsq_xg, 2 sum_g, 3 sumsq_g
    stats_block = spool.tile([128, 4], F32, name="stats_block")
    V.memset(stats_block[:], 0.0)

    # x_g and g live in DRAM between the R phase and the per-sample
    # P/M/E pass; inside each phase they occupy tag-shared SBUF tiles.
    def big_tiles(prefix):
        return [big.tile([128, W], F32, name=f"{prefix}_{c}")
                for c in range(CT)]

    # ---------------- helpers ---------------------------------------------
    def rstd_from_var(varcol):
        """[128,1] biased var -> 1/sqrt(var+eps), Newton-refined."""
        veps = nrm.tile([128, 1], F32, name="veps")
        V.tensor_scalar_add(out=veps[:], in0=varcol, scalar1=EPS)
        s0 = nrm.tile([128, 1], F32, name="s0")
        S.activation(s0[:], veps[:], AF.Sqrt)
        y0 = nrm.tile([128, 1], F32, name="y0")
        V.reciprocal(out=y0[:], in_=s0[:])
        t1 = nrm.tile([128, 1], F32, name="nt1")
        V.tensor_tensor(out=t1[:], in0=y0[:], in1=y0[:], op=ALU.mult)
        V.tensor_tensor(out=t1[:], in0=t1[:], in1=veps[:], op=ALU.mult)
        V.tensor_scalar(out=t1[:], in0=t1[:], scalar1=-0.5, scalar2=1.5,
                        op0=ALU.mult, op1=ALU.add)
        y1 = nrm.tile([128, 1], F32, name="ny1")
        V.tensor_tensor(out=y1[:], in0=y0[:], in1=t1[:], op=ALU.mult)
        return y1

    # ======================= R phase: residual block =======================
    with ExitStack() as rctx:
        conv_w = rctx.enter_context(tc.tile_pool(name="conv_w", bufs=1))
        natp = rctx.enter_context(tc.tile_pool(name="wnat", bufs=1))
        padp = rctx.enter_context(tc.tile_pool(name="padp", bufs=4))
        xre = rctx.enter_context(tc.tile_pool(name="xre", bufs=2))
        scr = rctx.enter_context(tc.tile_pool(name="scr", bufs=2))

        # conv weights rw[cout, cin, k] -> rwT[cin_p, k, cin_t, cout_t, :]
        def load_conv_weightT(name):
            wT = conv_w.tile([128, 3, CT, CT, 128], F32, name=f"{name}T")
            for co_t in range(CT):
                nat = natp.tile([128, C * 3], F32, name="wnat")
                nc.sync.dma_start(
                    out=nat[:],
                    in_=dram[name].ap().rearrange("a b c -> a (b c)")
                    [co_t * 128:(co_t + 1) * 128])
                for k in range(3):
                    for ci_t in range(CT):
                        ps = sm_psum.tile([128, 128], F32, name="smp")
                        src = nat[:, ci_t * 384 + k: (ci_t + 1) * 384: 3]
                        T.transpose(ps[:], src, ident[:])
                        V.tensor_copy(out=wT[:, k, ci_t, co_t, :], in_=ps[:])
            return wT

        rw1T = load_conv_weightT("rw1")
        rw2T = load_conv_weightT("rw2")

        # 1x1 weights [cout, cin, 1] -> wT[cin_p, cin_t, cout]
        def load_1x1_weightT(name):
            wT = qkv_w.tile([128, CT, C], F32, name=f"{name}T")
            for co_t in range(CT):
                nat = natp.tile([128, C * 3], F32, name="wnat")
                nc.sync.dma_start(
                    out=nat[:, 0:C],
                    in_=dram[name].ap().rearrange("a b c -> a (b c)")
                    [co_t * 128:(co_t + 1) * 128])
                for ci_t in range(CT):
                    ps = sm_psum.tile([128, 128], F32, name="smp")
                    T.transpose(ps[:], nat[:, ci_t * 128:(ci_t + 1) * 128],
                                ident[:])
                    V.tensor_copy(
                        out=wT[:, ci_t, co_t * 128:(co_t + 1) * 128],
                        in_=ps[:])
            return wT

        qwT = load_1x1_weightT("qw")
        kwT = load_1x1_weightT("kw")
        vwT = load_1x1_weightT("vw")

        def conv3(dst_cb, src_tiles, wT):
            """3-tap conv from padded [128, W+2] src tiles; dst_cb(co_t, ps)
            gets the accumulated [128, W] PSUM (bias not applied)."""
            for co_t in range(CT):
                ps = mm_psum.tile([128, W], F32, name="mm_ps")
                for jc in range(2):
                    idx = 0
                    for k in range(3):
                        for ci_t in range(CT):
                            T.matmul(ps[:, jc * 512:(jc + 1) * 512],
                                     lhsT=wT[:, k, ci_t, co_t, :],
                                     rhs=src_tiles[ci_t][:, jc * 512 + k:
                                                         jc * 512 + k + 512],
                                     start=(idx == 0), stop=(idx == 11))
                            idx += 1
                dst_cb(co_t, ps)

        def inorm_sin(dst, srcp, bias_col, out_accum=None):
            """dst <- sin(instance_norm(srcp + bias)); srcp: [128, W] PSUM."""
            t = scr.tile([128, W], F32, name="scrA")
            S.activation(t[:], srcp[:], AF.Identity, bias=bias_col)
            st = nrm.tile([128, 2, 6], F32, name="bn_st")
            V.bn_stats(st[:, 0, :], t[:, 0:512])
            V.bn_stats(st[:, 1, :], t[:, 512:1024])
            mv = nrm.tile([128, 2], F32, name="bn_mv")
            V.bn_aggr(mv[:], st[:])
            rstd = rstd_from_var(mv[:, 1:2])
            w = scr.tile([128, W], F32, name="scrB")
            V.tensor_scalar(out=w[:], in0=t[:], scalar1=mv[:, 0:1],
                            scalar2=rstd[:], op0=ALU.subtract, op1=ALU.mult)
            u = scr.tile([128, W], F32, name="scrA")
            V.tensor_scalar(out=u[:], in0=w[:], scalar1=INV2PI, scalar2=MAGIC,
                            op0=ALU.mult, op1=ALU.add)
            V.tensor_scalar(out=u[:], in0=u[:], scalar1=MAGIC, scalar2=None,
                            op0=ALU.subtract, op1=ALU.bypass)
            V.scalar_tensor_tensor(out=u[:], in0=u[:], scalar=-TWOPI,
                                   in1=w[:], op0=ALU.mult, op1=ALU.add)
            S.activation(dst, u[:], AF.Sin, accum_out=out_accum)

        for s in range(SPC):
            xp = [padp.tile([128, W + 2], F32, name="pad") for _ in range(CT)]
            for c in range(CT):
                V.memset(xp[c][:], 0.0)
                nc.sync.dma_start(
                    out=xp[c][:, 1:W + 1],
                    in_=dram["x"].ap()[s, c * 128:(c + 1) * 128, :])

            g1p = [padp.tile([128, W + 2], F32, name="pad") for _ in range(CT)]
            for c in range(CT):
                V.memset(g1p[c][:], 0.0)

            def c1_cb(co_t, ps):
                inorm_sin(g1p[co_t][:, 1:W + 1], ps, rb1b[:, co_t:co_t + 1])

            conv3(c1_cb, xp, rw1T)

            gloc = big_tiles("g")
            gsum = [None] * CT

            def c2_cb(co_t, ps):
                gsum[co_t] = nrm.tile([128, 1], F32, name=f"gsum{co_t}")
                inorm_sin(gloc[co_t][:], ps, rb2b[:, co_t:co_t + 1],
                          out_accum=gsum[co_t][:])

            conv3(c2_cb, g1p, rw2T)

            # x_g = x + g (x re-streamed, in place in the stream tile),
            # sum/sumsq stats, then spill x_g and g to DRAM
            for c in range(CT):
                xt = xre.tile([128, W], F32, name="xre")
                nc.sync.dma_start(
                    out=xt[:],
                    in_=dram["x"].ap()[s, c * 128:(c + 1) * 128, :])
                xs1 = nrm.tile([128, 1], F32, name="xs1")
                V.scalar_tensor_tensor(out=xt[:], in0=xt[:],
                                       scalar=0.0, in1=gloc[c][:],
                                       op0=ALU.add, op1=ALU.add,
                                       accum_out=xs1[:])
                xs2 = nrm.tile([128, 1], F32, name="xs2")
                sq = scr.tile([128, W], F32, name="scrB")
                V.scalar_tensor_tensor(out=sq[:], in0=xt[:], scalar=0.0,
                                       in1=xt[:], op0=ALU.add, op1=ALU.mult,
                                       accum_out=xs2[:])
                gs2 = nrm.tile([128, 1], F32, name="gs2")
                sq2 = scr.tile([128, W], F32, name="scrB")
                V.scalar_tensor_tensor(out=sq2[:], in0=gloc[c][:], scalar=0.0,
                                       in1=gloc[c][:], op0=ALU.add,
                                       op1=ALU.mult, accum_out=gs2[:])
                nc.sync.dma_start(
                    out=dram["xg_d"].ap()[s, c * 128:(c + 1) * 128, :],
                    in_=xt[:])
                nc.sync.dma_start(
                    out=dram["g_d"].ap()[s, c * 128:(c + 1) * 128, :],
                    in_=gloc[c][:])
                V.tensor_tensor(out=stats_block[:, 0:1],
                                in0=stats_block[:, 0:1], in1=xs1[:],
                                op=ALU.add)
                V.tensor_tensor(out=stats_block[:, 1:2],
                                in0=stats_block[:, 1:2], in1=xs2[:],
                                op=ALU.add)
                V.tensor_tensor(out=stats_block[:, 2:3],
                                in0=stats_block[:, 2:3], in1=gsum[c][:],
                                op=ALU.add)
                V.tensor_tensor(out=stats_block[:, 3:4],
                                in0=stats_block[:, 3:4], in1=gs2[:],
                                op=ALU.add)

    # ====================== AllReduce of the 4 sums ========================
    ps4 = sm_psum.tile([128, 128], F32, name="smp")
    T.matmul(ps4[:1, 0:4], lhsT=ones128[:], rhs=stats_block[:],
             start=True, stop=True)
    cc_sb = spool.tile([1, 4], F32, name="cc_sb")
    V.tensor_copy(out=cc_sb[:], in_=ps4[:1, 0:4])
    nc.sync.dma_start(out=dram["cc_in"].ap(), in_=cc_sb[:])
    nc.gpsimd.collective_compute(
        "AllReduce", ALU.add,
        replica_groups=[list(range(N_CORES))],
        ins=[dram["cc_in"].ap()],
        outs=[dram["cc_out"].ap()],
    )
    gstat = spool.tile([128, 4], F32, name="gstat")
    bcast = bass.AP(tensor=dram["cc_out"], offset=0, ap=[[0, 128], [1, 4]])
    nc.sync.dma_start(out=gstat[:], in_=bcast)

    # m = S1/N ; var = (S2 - S1^2/N)/(N-1) ; rs = 1/sqrt(var) (Newton)
    def mean_rs(s1col, s2col, tag):
        m = spool.tile([128, 1], F32, name=f"m_{tag}")
        V.tensor_scalar_mul(out=m[:], in0=s1col, scalar1=1.0 / NTOT)
        t = spool.tile([128, 1], F32, name=f"v_{tag}")
        V.tensor_tensor(out=t[:], in0=s1col, in1=m[:], op=ALU.mult)
        V.tensor_scalar_mul(out=t[:], in0=t[:], scalar1=-1.0)
        V.tensor_tensor(out=t[:], in0=t[:], in1=s2col, op=ALU.add)
        V.tensor_scalar_mul(out=t[:], in0=t[:], scalar1=1.0 / (NTOT - 1.0))
        sq = spool.tile([128, 1], F32, name=f"sq_{tag}")
        S.activation(sq[:], t[:], AF.Sqrt)
        y0 = spool.tile([128, 1], F32, name=f"y0_{tag}")
        V.reciprocal(out=y0[:], in_=sq[:])
        t2 = spool.tile([128, 1], F32, name=f"t2_{tag}")
        V.tensor_tensor(out=t2[:], in0=y0[:], in1=y0[:], op=ALU.mult)
        V.tensor_tensor(out=t2[:], in0=t2[:], in1=t[:], op=ALU.mult)
        V.tensor_scalar(out=t2[:], in0=t2[:], scalar1=-0.5, scalar2=1.5,
                        op0=ALU.mult, op1=ALU.add)
        V.tensor_tensor(out=t2[:], in0=y0[:], in1=t2[:], op=ALU.mult)
        return m, t2

    m_xg, rs_xg = mean_rs(gstat[:, 0:1], gstat[:, 1:2], "xg")
    m_g, rs_g = mean_rs(gstat[:, 2:3], gstat[:, 3:4], "g")
    alpha = spool.tile([128, 1], F32, name="alpha")
    V.tensor_tensor(out=alpha[:], in0=rs_xg[:], in1=rs_g[:], op=ALU.mult)
    negalpha = spool.tile([128, 1], F32, name="negalpha")
    V.tensor_scalar_mul(out=negalpha[:], in0=alpha[:], scalar1=-1.0)

    # =================== P/M/E phases, one sample at a time ================
    with ExitStack() as ectx:
        proj = ectx.enter_context(tc.tile_pool(name="proj", bufs=1))
        m2p = ectx.enter_context(tc.tile_pool(name="m2p", bufs=1))
        attp = ectx.enter_context(tc.tile_pool(name="attp", bufs=1))
        strm = ectx.enter_context(tc.tile_pool(name="strm", bufs=2))
        epool = ectx.enter_context(tc.tile_pool(name="epool", bufs=2))

        def proj_1x1(src_tiles, wT, bias_cols, prefix):
            res = []
            for co_t in range(CT):
                ps = mm_psum.tile([128, W], F32, name="mm_ps")
                for jc in range(2):
                    for ci_t in range(CT):
                        T.matmul(
                            ps[:, jc * 512:(jc + 1) * 512],
                            lhsT=wT[:, ci_t, co_t * 128:(co_t + 1) * 128],
                            rhs=src_tiles[ci_t][:, jc * 512:(jc + 1) * 512],
                            start=(ci_t == 0), stop=(ci_t == CT - 1))
                t = proj.tile([128, W], F32, name=f"{prefix}{co_t}")
                S.activation(t[:], ps[:], AF.Identity,
                             bias=bias_cols[:, co_t:co_t + 1])
                res.append(t)
            return res

        for s in range(SPC):
            # ---- reload x_g, g ----
            xgs = big_tiles("xg")
            ggs = big_tiles("g2")
            for c in range(CT):
                nc.sync.dma_start(
                    out=xgs[c][:],
                    in_=dram["xg_d"].ap()[s, c * 128:(c + 1) * 128, :])
                nc.sync.dma_start(
                    out=ggs[c][:],
                    in_=dram["g_d"].ap()[s, c * 128:(c + 1) * 128, :])

            # ---- pq -> center x_g -> M1 -> DRAM ----
            pq = proj_1x1(xgs, qwT, qbb, "pj")
            for c in range(CT):
                V.tensor_scalar(out=xgs[c][:], in0=xgs[c][:],
                                scalar1=m_xg[:], scalar2=None,
                                op0=ALU.subtract, op1=ALU.bypass)
            for kt in range(KT):
                ps = mm_psum.tile([128, W], F32, name="mm_ps")
                for jc in range(2):
                    for ci_t in range(CT):
                        T.matmul(ps[:, jc * 512:(jc + 1) * 512],
                                 lhsT=xgs[ci_t][:, kt * 128:(kt + 1) * 128],
                                 rhs=pq[ci_t][:, jc * 512:(jc + 1) * 512],
                                 start=(ci_t == 0), stop=(ci_t == CT - 1))
                t = outbuf.tile([128, W], F32, name="ob")
                V.tensor_copy(out=t[:], in_=ps[:])
                nc.sync.dma_start(
                    out=dram["m1"].ap()[s, kt * 128:(kt + 1) * 128, :],
                    in_=t[:])

            # ---- pk, pvT (raw g) -> center g -> M2 -> SBUF ----
            pk = proj_1x1(ggs, kwT, kbb, "pj")
            for kt in range(KT):
                ps = mm_psum.tile([128, W], F32, name="mm_ps")
                for ci_t in range(CT):
                    T.matmul(ps[:, 0:C],
                             lhsT=ggs[ci_t][:, kt * 128:(kt + 1) * 128],
                             rhs=vwT[:, ci_t, :],
                             start=(ci_t == 0), stop=False)
                T.matmul(ps[:, 0:C], lhsT=ones1[:], rhs=vb_row[:],
                         start=False, stop=True)
                t = outbuf.tile([128, C], F32, name="ob")
                V.tensor_copy(out=t[:], in_=ps[:, 0:C])
                nc.sync.dma_start(
                    out=dram["pvT"].ap()[s, kt * 128:(kt + 1) * 128, :],
                    in_=t[:])
            for c in range(CT):
                V.tensor_scalar(out=ggs[c][:], in0=ggs[c][:],
                                scalar1=m_g[:], scalar2=None,
                                op0=ALU.subtract, op1=ALU.bypass)
            m2 = []
            for kt in range(KT):
                ps = mm_psum.tile([128, W], F32, name="mm_ps")
                for jc in range(2):
                    for ci_t in range(CT):
                        T.matmul(ps[:, jc * 512:(jc + 1) * 512],
                                 lhsT=ggs[ci_t][:, kt * 128:(kt + 1) * 128],
                                 rhs=pk[ci_t][:, jc * 512:(jc + 1) * 512],
                                 start=(ci_t == 0), stop=(ci_t == CT - 1))
                mt = m2p.tile([128, W], F32, name=f"m2_{kt}")
                V.tensor_copy(out=mt[:], in_=ps[:])
                m2.append(mt)

            # ---- E(s): energy -> softmax -> att^T -> out ----
            attT = [attp.tile([128, W], F32, name=f"attT_{kt}")
                    for kt in range(KT)]
            for it in range(KT):
                ps = mm_psum.tile([128, W], F32, name="mm_ps")
                m1blk = []
                for kt in range(KT):
                    blk = strm.tile([128, 128], F32, name=f"blk{kt}")
                    nc.sync.dma_start(
                        out=blk[:],
                        in_=dram["m1"].ap()[s, kt * 128:(kt + 1) * 128,
                                            it * 128:(it + 1) * 128])
                    m1blk.append(blk)
                for jc in range(2):
                    for kt in range(KT):
                        T.matmul(ps[:, jc * 512:(jc + 1) * 512],
                                 lhsT=m1blk[kt][:],
                                 rhs=m2[kt][:, jc * 512:(jc + 1) * 512],
                                 start=(kt == 0), stop=(kt == KT - 1))
                rowmax = nrm.tile([128, 1], F32, name="rowmax")
                V.tensor_reduce(out=rowmax[:], in_=ps[:], axis=AX.X,
                                op=ALU.max)
                nb = nrm.tile([128, 1], F32, name="negb")
                V.tensor_tensor(out=nb[:], in0=rowmax[:], in1=negalpha[:],
                                op=ALU.mult)
                e = epool.tile([128, W], F32, name="e_t")
                rowsum = nrm.tile([128, 1], F32, name="rowsum")
                S.activation(e[:], ps[:], AF.Exp, bias=nb[:], scale=alpha[:],
                             accum_out=rowsum[:])
                rs = nrm.tile([128, 1], F32, name="rs")
                V.reciprocal(out=rs[:], in_=rowsum[:])
                V.tensor_scalar_mul(out=e[:], in0=e[:], scalar1=rs[:])
                for kt in range(KT):
                    tp = sm_psum.tile([128, 128], F32, name="smp")
                    T.transpose(tp[:], e[:, kt * 128:(kt + 1) * 128],
                                ident[:])
                    V.tensor_copy(out=attT[kt][:, it * 128:(it + 1) * 128],
                                  in_=tp[:])
            for ct in range(CT):
                ps = mm_psum.tile([128, W], F32, name="mm_ps")
                pvblk = []
                for kt in range(KT):
                    blk = strm.tile([128, 128], F32, name=f"blk{kt}")
                    nc.sync.dma_start(
                        out=blk[:],
                        in_=dram["pvT"].ap()[s, kt * 128:(kt + 1) * 128,
                                             ct * 128:(ct + 1) * 128])
                    pvblk.append(blk)
                for jc in range(2):
                    for kt in range(KT):
                        T.matmul(ps[:, jc * 512:(jc + 1) * 512],
                                 lhsT=pvblk[kt][:],
                                 rhs=attT[kt][:, jc * 512:(jc + 1) * 512],
                                 start=(kt == 0), stop=(kt == KT - 1))
                t = outbuf.tile([128, W], F32, name="ob")
                V.tensor_copy(out=t[:], in_=ps[:])
                nc.sync.dma_start(
                    out=dram["y"].ap()[s, ct * 128:(ct + 1) * 128, :],
                    in_=t[:])


def _build():
    nc = bass.Bass("TRN2", target_bir_lowering=False, debug=False,
                   num_devices=N_CORES)
    dram = {}
    dram["x"] = nc.dram_tensor("x", [SPC, C, W], F32, kind="ExternalInput")
    for nm, shp in [("qw", [C, C, 1]), ("kw", [C, C, 1]), ("vw", [C, C, 1]),
                    ("rw1", [C, C, 3]), ("rw2", [C, C, 3])]:
        dram[nm] = nc.dram_tensor(nm, shp, F32, kind="ExternalInput")
    for nm in ["qb", "kb", "vb", "rb1", "rb2"]:
        dram[nm] = nc.dram_tensor(nm, [C], F32, kind="ExternalInput")
    dram["y"] = nc.dram_tensor("y", [SPC, C, W], F32, kind="ExternalOutput")
    dram["m1"] = nc.dram_tensor("m1", [SPC, W, W], F32)
    dram["xg_d"] = nc.dram_tensor("xg_d", [SPC, C, W], F32)
    dram["g_d"] = nc.dram_tensor("g_d", [SPC, C, W], F32)
    dram["pvT"] = nc.dram_tensor("pvT", [SPC, W, C], F32)
    dram["cc_in"] = nc.dram_tensor("cc_in", [1, 4], F32)
    dram["cc_out"] = nc.dram_tensor("cc_out", [1, 4], F32,
                                    addr_space="Shared")

    with tile.TileContext(nc) as tc:
        with ExitStack() as ctx:
            _emit(nc, tc, ctx, dram)
    _split_multiwait(nc)
    return nc


_NC_CACHE = {}


def kernel(**inputs):
    if "nc" not in _NC_CACHE:
        _NC_CACHE["nc"] = _build()
    nc = _NC_CACHE["nc"]
    x = np.ascontiguousarray(np.asarray(inputs["x"], dtype=np.float32))
    common = {}
    for nm in ["qw", "kw", "vw", "rw1", "rw2", "qb", "kb", "vb",
               "rb1", "rb2"]:
        common[nm] = np.ascontiguousarray(
            np.asarray(inputs[nm], dtype=np.float32))
    in_maps = []
    for core in range(N_CORES):
        m = dict(common)
        m["x"] = np.ascontiguousarray(x[core * SPC:(core + 1) * SPC])
        in_maps.append(m)
    res = run_bass_kernel_spmd(nc, in_maps, core_ids=list(range(N_CORES)))
    y = np.concatenate([r["y"] for r in res.results], axis=0)
    return y


# revision 17
# speedup vs baseline: 1.2249x; 1.2249x over previous
"""Trainium2 Bass kernel for nn_CCAR_11579231830663 (dense_transformer).

Data-parallel over batch: 16 samples -> 8 NeuronCores x 2 samples. The global
z-score mean/std of x_g and g are the only cross-core terms; each core
all-reduces 4 scalar sums (sum/sumsq of x_g and g) on-device.

Per sample (C=512, W=1024):
  g   = sin(IN(conv3(x, rw1)+rb1)); g = sin(IN(conv3(g, rw2)+rb2))
  x_g = x + g
  qe  = zscore(x_g)^T . pq,  pq = qw@x_g + qb     (z-score over ALL of x_g)
  ke  = zscore(g)^T  . pk,  pk = kw@g + kb
  energy = qe @ ke ; att = softmax(energy); out = (vw@g+vb) @ att^T
The z-score is folded out: center x_g/g by the global means, compute raw
bilinear forms M1 = xc^T@pq (lhsT for energy), M2 = gc^T@pk (rhs), and fold
alpha = 1/(s_xg*s_g) into the exp: softmax(a*E) = exp(a*(E-rowmax))/sum, a>0.
"""
import sys
sys.path.insert(0, '/opt/trn_rl_repo')

import numpy as np
from contextlib import ExitStack

import concourse.bass as bass
import concourse.tile as tile
from concourse import mybir
from concourse.masks import make_identity
from concourse.bass_utils import run_bass_kernel_spmd

F32 = mybir.dt.float32
AF = mybir.ActivationFunctionType
ALU = mybir.AluOpType
AX = mybir.AxisListType

N_CORES = 8
B, C, W = 16, 512, 1024
SPC = B // N_CORES      # samples per core
CT = C // 128           # channel tiles
KT = W // 128           # width 128-tiles
EPS = 1e-5
NTOT = float(B * C * W)

DEBUG_DUMP = False
MAGIC = 12582912.0       # 1.5*2^23 fp32 round-to-nearest-int magic
TWOPI = float(2 * np.pi)
INV2PI = float(1.0 / (2 * np.pi))

# ---------------------------------------------------------------------------
# walrus workaround: this container's walrus accepts only a limited number of
# sync waits per instruction; Tile can aggregate more (e.g. the tail drain).
# Split excess waits onto same-engine NOPs placed just before the instruction.
_uid = [0]


def _split_multiwait(nc, limit=1):
    for f in nc.m.functions:
        for bb in f.blocks:
            insts = list(bb.instructions)
            out = []
            changed = False
            for inst in insts:
                si = inst.sync_info
                waits = list(si.on_wait) if si is not None and si.on_wait else []
                if len(waits) > limit:
                    changed = True
                    excess, keep = waits[:-limit], waits[-limit:]
                    si.on_wait = keep
                    inst.sync_info = si
                    for i in range(0, len(excess), limit):
                        chunk = excess[i:i + limit]
                        _uid[0] += 1
                        nop = mybir.InstNoOp(
                            name=f"I-waitsplit-{_uid[0]}", ins=[], outs=[])
                        nop.engine = inst.engine
                        nop.sync_info = mybir.SyncInfo(
                            on_wait=chunk, on_update=[])
                        out.append(nop)
                out.append(inst)
            if changed:
                bb.instructions = out


# ---------------------------------------------------------------------------
def _emit(nc, tc, ctx, dram):
    V = nc.vector
    S = nc.scalar
    T = nc.tensor

    # ---------------- outer pools (small / long-lived) --------------------
    singles = ctx.enter_context(tc.tile_pool(name="singles", bufs=1))
    spool = ctx.enter_context(tc.tile_pool(name="spool", bufs=1))
    nrm = ctx.enter_context(tc.tile_pool(name="nrm", bufs=2))
    outbuf = ctx.enter_context(tc.tile_pool(name="outbuf", bufs=2))
    qkv_w = ctx.enter_context(tc.tile_pool(name="qkv_w", bufs=1))
    big = ctx.enter_context(tc.tile_pool(name="big", bufs=1))
    mm_psum = ctx.enter_context(
        tc.tile_pool(name="mm_psum", bufs=3, space="PSUM"))
    sm_psum = ctx.enter_context(
        tc.tile_pool(name="sm_psum", bufs=2, space="PSUM"))

    ident = singles.tile([128, 128], F32, name="ident")
    make_identity(nc, ident[:])
    ones1 = singles.tile([1, 128], F32, name="ones1")
    V.memset(ones1[:], 1.0)
    ones128 = singles.tile([128, 1], F32, name="ones128")
    V.memset(ones128[:], 1.0)

    def load_bias_cols(name):
        t = singles.tile([128, CT], F32, name=f"{name}_cols")
        src = dram[name].ap().rearrange("(t p) -> p t", p=128)
        nc.sync.dma_start(out=t[:], in_=src)
        return t

    rb1b = load_bias_cols("rb1")
    rb2b = load_bias_cols("rb2")
    qbb = load_bias_cols("qb")
    kbb = load_bias_cols("kb")
    vb_row = singles.tile([1, C], F32, name="vb_row")
    nc.sync.dma_start(out=vb_row[:], in_=dram["vb"].ap()[None, :])
    qb_row = singles.tile([1, C], F32, name="qb_row")
    nc.sync.dma_start(out=qb_row[:], in_=dram["qb"].ap()[None, :])

    # columns: 0 sum_xg, 1 sumsq_xg, 2 sum_g, 3 sumsq_g
    stats_block = spool.tile([128, 4], F32, name="stats_block")
    V.memset(stats_block[:], 0.0)

    # x_g and g live in DRAM between the R phase and the per-sample
    # P/M/E pass; inside each phase they occupy tag-shared SBUF tiles.
    def big_tiles(prefix):
        return [big.tile([128, W], F32, name=f"{prefix}_{c}")
                for c in range(CT)]

    # ---------------- helpers ---------------------------------------------
    def rstd_from_var(varcol):
        """[128,1] biased var -> 1/sqrt(var+eps), Newton-refined."""
        veps = nrm.tile([128, 1], F32, name="veps")
        V.tensor_scalar_add(out=veps[:], in0=varcol, scalar1=EPS)
        s0 = nrm.tile([128, 1], F32, name="s0")
        S.activation(s0[:], veps[:], AF.Sqrt)
        y0 = nrm.tile([128, 1], F32, name="y0")
        V.reciprocal(out=y0[:], in_=s0[:])
        t1 = nrm.tile([128, 1], F32, name="nt1")
        V.tensor_tensor(out=t1[:], in0=y0[:], in1=y0[:], op=ALU.mult)
        V.tensor_tensor(out=t1[:], in0=t1[:], in1=veps[:], op=ALU.mult)
        V.tensor_scalar(out=t1[:], in0=t1[:], scalar1=-0.5, scalar2=1.5,
                        op0=ALU.mult, op1=ALU.add)
        y1 = nrm.tile([128, 1], F32, name="ny1")
        V.tensor_tensor(out=y1[:], in0=y0[:], in1=t1[:], op=ALU.mult)
        return y1

    # ======================= R phase: residual block =======================
    with ExitStack() as rctx:
        conv_w = rctx.enter_context(tc.tile_pool(name="conv_w", bufs=1))
        natp = rctx.enter_context(tc.tile_pool(name="wnat", bufs=1))
        padp = rctx.enter_context(tc.tile_pool(name="padp", bufs=4))
        xre = rctx.enter_context(tc.tile_pool(name="xre", bufs=2))
        scr = rctx.enter_context(tc.tile_pool(name="scr", bufs=2))

        # conv weights rw[cout, cin, k] -> rwT[cin_p, k, cin_t, cout_t, :]
        def load_conv_weightT(name):
            wT = conv_w.tile([128, 3, CT, CT, 128], F32, name=f"{name}T")
            for co_t in range(CT):
                nat = natp.tile([128, C * 3], F32, name="wnat")
                nc.sync.dma_start(
                    out=nat[:],
                    in_=dram[name].ap().rearrange("a b c -> a (b c)")
                    [co_t * 128:(co_t + 1) * 128])
                for k in range(3):
                    for ci_t in range(CT):
                        ps = sm_psum.tile([128, 128], F32, name="smp")
                        src = nat[:, ci_t * 384 + k: (ci_t + 1) * 384: 3]
                        T.transpose(ps[:], src, ident[:])
                        V.tensor_copy(out=wT[:, k, ci_t, co_t, :], in_=ps[:])
            return wT

        rw1T = load_conv_weightT("rw1")
        rw2T = load_conv_weightT("rw2")

        # 1x1 weights [cout, cin, 1] -> wT[cin_p, cin_t, cout]
        def load_1x1_weightT(name):
            wT = qkv_w.tile([128, CT, C], F32, name=f"{name}T")
            for co_t in range(CT):
                nat = natp.tile([128, C * 3], F32, name="wnat")
                nc.sync.dma_start(
                    out=nat[:, 0:C],
                    in_=dram[name].ap().rearrange("a b c -> a (b c)")
                    [co_t * 128:(co_t + 1) * 128])
                for ci_t in range(CT):
                    ps = sm_psum.tile([128, 128], F32, name="smp")
                    T.transpose(ps[:], nat[:, ci_t * 128:(ci_t + 1) * 128],
                                ident[:])
                    V.tensor_copy(
                        out=wT[:, ci_t, co_t * 128:(co_t + 1) * 128],
                        in_=ps[:])
            return wT

        qwT = load_1x1_weightT("qw")
        kwT = load_1x1_weightT("kw")
        vwT = load_1x1_weightT("vw")

        def conv3(dst_cb, src_tiles, wT):
            """3-tap conv from padded [128, W+2] src tiles; dst_cb(co_t, ps)
            gets the accumulated [128, W] PSUM (bias not applied)."""
            for co_t in range(CT):
                ps = mm_psum.tile([128, W], F32, name="mm_ps")
                for jc in range(2):
                    idx = 0
                    for k in range(3):
                        for ci_t in range(CT):
                            T.matmul(ps[:, jc * 512:(jc + 1) * 512],
                                     lhsT=wT[:, k, ci_t, co_t, :],
                                     rhs=src_tiles[ci_t][:, jc * 512 + k:
                                                         jc * 512 + k + 512],
                                     start=(idx == 0), stop=(idx == 11))
                            idx += 1
                dst_cb(co_t, ps)

        def inorm_sin(dst, srcp, bias_col, out_accum=None):
            """dst <- sin(instance_norm(srcp + bias)); srcp: [128, W] PSUM."""
            t = scr.tile([128, W], F32, name="scrA")
            S.activation(t[:], srcp[:], AF.Identity, bias=bias_col)
            st = nrm.tile([128, 2, 6], F32, name="bn_st")
            V.bn_stats(st[:, 0, :], t[:, 0:512])
            V.bn_stats(st[:, 1, :], t[:, 512:1024])
            mv = nrm.tile([128, 2], F32, name="bn_mv")
            V.bn_aggr(mv[:], st[:])
            rstd = rstd_from_var(mv[:, 1:2])
            w = scr.tile([128, W], F32, name="scrB")
            V.tensor_scalar(out=w[:], in0=t[:], scalar1=mv[:, 0:1],
                            scalar2=rstd[:], op0=ALU.subtract, op1=ALU.mult)
            u = scr.tile([128, W], F32, name="scrA")
            V.tensor_scalar(out=u[:], in0=w[:], scalar1=INV2PI, scalar2=MAGIC,
                            op0=ALU.mult, op1=ALU.add)
            V.tensor_scalar(out=u[:], in0=u[:], scalar1=MAGIC, scalar2=None,
                            op0=ALU.subtract, op1=ALU.bypass)
            V.scalar_tensor_tensor(out=u[:], in0=u[:], scalar=-TWOPI,
                                   in1=w[:], op0=ALU.mult, op1=ALU.add)
            S.activation(dst, u[:], AF.Sin, accum_out=out_accum)

        for s in range(SPC):
            xp = [padp.tile([128, W + 2], F32, name="pad") for _ in range(CT)]
            for c in range(CT):
                V.memset(xp[c][:], 0.0)
                nc.sync.dma_start(
                    out=xp[c][:, 1:W + 1],
                    in_=dram["x"].ap()[s, c * 128:(c + 1) * 128, :])

            g1p = [padp.tile([128, W + 2], F32, name="pad") for _ in range(CT)]
            for c in range(CT):
                V.memset(g1p[c][:], 0.0)

            def c1_cb(co_t, ps):
                inorm_sin(g1p[co_t][:, 1:W + 1], ps, rb1b[:, co_t:co_t + 1])

            conv3(c1_cb, xp, rw1T)

            gloc = big_tiles("g")
            gsum = [None] * CT

            def c2_cb(co_t, ps):
                gsum[co_t] = nrm.tile([128, 1], F32, name=f"gsum{co_t}")
                inorm_sin(gloc[co_t][:], ps, rb2b[:, co_t:co_t + 1],
                          out_accum=gsum[co_t][:])

            conv3(c2_cb, g1p, rw2T)

            # x_g = x + g (x re-streamed, in place in the stream tile),
            # sum/sumsq stats, then spill x_g and g to DRAM
            for c in range(CT):
                xt = xre.tile([128, W], F32, name="xre")
                nc.sync.dma_start(
                    out=xt[:],
                    in_=dram["x"].ap()[s, c * 128:(c + 1) * 128, :])
                xs1 = nrm.tile([128, 1], F32, name="xs1")
                V.scalar_tensor_tensor(out=xt[:], in0=xt[:],
                                       scalar=0.0, in1=gloc[c][:],
                                       op0=ALU.add, op1=ALU.add,
                                       accum_out=xs1[:])
                xs2 = nrm.tile([128, 1], F32, name="xs2")
                sq = scr.tile([128, W], F32, name="scrB")
                V.scalar_tensor_tensor(out=sq[:], in0=xt[:], scalar=0.0,
                                       in1=xt[:], op0=ALU.add, op1=ALU.mult,
                                       accum_out=xs2[:])
                gs2 = nrm.tile([128, 1], F32, name="gs2")
                sq2 = scr.tile([128, W], F32, name="scrB")
                V.scalar_tensor_tensor(out=sq2[:], in0=gloc[c][:], scalar=0.0,
                                       in1=gloc[c][:], op0=ALU.add,
                                       op1=ALU.mult, accum_out=gs2[:])
                nc.sync.dma_start(
                    out=dram["xg_d"].ap()[s, c * 128:(c + 1) * 128, :],
                    in_=xt[:])
                nc.sync.dma_start(
                    out=dram["g_d"].ap()[s, c * 128:(c + 1) * 128, :],
                    in_=gloc[c][:])
                V.tensor_tensor(out=stats_block[:, 0:1],
                                in0=stats_block[:, 0:1], in1=xs1[:],
                                op=ALU.add)
                V.tensor_tensor(out=stats_block[:, 1:2],
                                in0=stats_block[:, 1:2], in1=xs2[:],
                                op=ALU.add)
                V.tensor_tensor(out=stats_block[:, 2:3],
                                in0=stats_block[:, 2:3], in1=gsum[c][:],
                                op=ALU.add)
                V.tensor_tensor(out=stats_block[:, 3:4],
                                in0=stats_block[:, 3:4], in1=gs2[:],
                                op=ALU.add)

    # ====================== AllReduce of the 4 sums ========================
    ps4 = sm_psum.tile([128, 128], F32, name="smp")
    T.matmul(ps4[:1, 0:4], lhsT=ones128[:], rhs=stats_block[:],
             start=True, stop=True)
    cc_sb = spool.tile([1, 4], F32, name="cc_sb")
    V.tensor_copy(out=cc_sb[:], in_=ps4[:1, 0:4])
    nc.sync.dma_start(out=dram["cc_in"].ap(), in_=cc_sb[:])
    nc.gpsimd.collective_compute(
        "AllReduce", ALU.add,
        replica_groups=[list(range(N_CORES))],
        ins=[dram["cc_in"].ap()],
        outs=[dram["cc_out"].ap()],
    )
    gstat = spool.tile([128, 4], F32, name="gstat")
    bcast = bass.AP(tensor=dram["cc_out"], offset=0, ap=[[0, 128], [1, 4]])
    nc.sync.dma_start(out=gstat[:], in_=bcast)

    # m = S1/N ; var = (S2 - S1^2/N)/(N-1) ; rs = 1/sqrt(var) (Newton)
    def mean_rs(s1col, s2col, tag):
        m = spool.tile([128, 1], F32, name=f"m_{tag}")
        V.tensor_scalar_mul(out=m[:], in0=s1col, scalar1=1.0 / NTOT)
        t = spool.tile([128, 1], F32, name=f"v_{tag}")
        V.tensor_tensor(out=t[:], in0=s1col, in1=m[:], op=ALU.mult)
        V.tensor_scalar_mul(out=t[:], in0=t[:], scalar1=-1.0)
        V.tensor_tensor(out=t[:], in0=t[:], in1=s2col, op=ALU.add)
        V.tensor_scalar_mul(out=t[:], in0=t[:], scalar1=1.0 / (NTOT - 1.0))
        sq = spool.tile([128, 1], F32, name=f"sq_{tag}")
        S.activation(sq[:], t[:], AF.Sqrt)
        y0 = spool.tile([128, 1], F32, name=f"y0_{tag}")
        V.reciprocal(out=y0[:], in_=sq[:])
        t2 = spool.tile([128, 1], F32, name=f"t2_{tag}")
        V.tensor_tensor(out=t2[:], in0=y0[:], in1=y0[:], op=ALU.mult)
        V.tensor_tensor(out=t2[:], in0=t2[:], in1=t[:], op=ALU.mult)
        V.tensor_scalar(out=t2[:], in0=t2[:], scalar1=-0.5, scalar2=1.5,
                        op0=ALU.mult, op1=ALU.add)
        V.tensor_tensor(out=t2[:], in0=y0[:], in1=t2[:], op=ALU.mult)
        return m, t2

    m_xg, rs_xg = mean_rs(gstat[:, 0:1], gstat[:, 1:2], "xg")
    m_g, rs_g = mean_rs(gstat[:, 2:3], gstat[:, 3:4], "g")
    alpha = spool.tile([128, 1], F32, name="alpha")
    V.tensor_tensor(out=alpha[:], in0=rs_xg[:], in1=rs_g[:], op=ALU.mult)
    negalpha = spool.tile([128, 1], F32, name="negalpha")
    V.tensor_scalar_mul(out=negalpha[:], in0=alpha[:], scalar1=-1.0)

    # =================== P/E phases, one sample at a time ==================
    # Reference algebra: energy = a * pq^T @ (xc @ gc^T) @ pk, a = 1/(sx*sg)
    #   MT[c',c] = sum_k gc[c',k] xc[c,k]      (via gcT, xcT: k-partitioned)
    #   Mp[c,j]  = sum_c' MT[c',c] pk[c',j]
    #   E[i,j]   = sum_c pq[c,i] Mp[c,j]
    with ExitStack() as ectx:
        pme = ectx.enter_context(tc.tile_pool(name="pme", bufs=1))
        attp = ectx.enter_context(tc.tile_pool(name="attp", bufs=1))
        strm = ectx.enter_context(tc.tile_pool(name="strm", bufs=2))

        def projT(src_tiles, wT, b_row, prefix):
            """proj^T[k, c] = sum_ci src[ci, k] w[c, ci] + b[c]"""
            res = []
            for kt in range(KT):
                ps = mm_psum.tile([128, W], F32, name="mm_ps")
                for ci_t in range(CT):
                    T.matmul(ps[:, 0:C],
                             lhsT=src_tiles[ci_t][:, kt * 128:(kt + 1) * 128],
                             rhs=wT[:, ci_t, :],
                             start=(ci_t == 0), stop=False)
                T.matmul(ps[:, 0:C], lhsT=ones1[:], rhs=b_row[:],
                         start=False, stop=True)
                t = pme.tile([128, C], F32, name=f"{prefix}{kt}")
                V.tensor_copy(out=t[:], in_=ps[:, 0:C])
                res.append(t)
            return res

        def proj_col(src_tiles, wT, bias_cols, prefix, spill=None):
            """proj[c, j] natural column form; optionally DMA to DRAM
            (spilled projections use rotating outbuf slots)."""
            res = []
            for co_t in range(CT):
                ps = mm_psum.tile([128, W], F32, name="mm_ps")
                for jc in range(2):
                    for ci_t in range(CT):
                        T.matmul(
                            ps[:, jc * 512:(jc + 1) * 512],
                            lhsT=wT[:, ci_t, co_t * 128:(co_t + 1) * 128],
                            rhs=src_tiles[ci_t][:, jc * 512:(jc + 1) * 512],
                            start=(ci_t == 0), stop=(ci_t == CT - 1))
                if spill is not None:
                    t = outbuf.tile([128, W], F32, name="ob")
                else:
                    t = pme.tile([128, W], F32, name=f"{prefix}{co_t}")
                S.activation(t[:], ps[:], AF.Identity,
                             bias=bias_cols[:, co_t:co_t + 1])
                if spill is not None:
                    nc.sync.dma_start(
                        out=dram[spill].ap()[co_t * 128:(co_t + 1) * 128, :],
                        in_=t[:])
                res.append(t)
            return res

        for s in range(SPC):
            # ---- reload x_g, g ----
            xgs = big_tiles("xg")
            ggs = big_tiles("g")
            for c in range(CT):
                nc.sync.dma_start(
                    out=xgs[c][:],
                    in_=dram["xg_d"].ap()[s, c * 128:(c + 1) * 128, :])
                nc.sync.dma_start(
                    out=ggs[c][:],
                    in_=dram["g_d"].ap()[s, c * 128:(c + 1) * 128, :])

            # ---- projections on raw x_g / g; pq spilled to DRAM ----
            proj_col(xgs, qwT, qbb, "pj", spill="pq_d")
            pk = proj_col(ggs, kwT, kbb, "pk")
            pvT = projT(ggs, vwT, vb_row, "pvT")

            # ---- center in place (needs the AllReduce result) ----
            for c in range(CT):
                V.tensor_scalar(out=xgs[c][:], in0=xgs[c][:],
                                scalar1=m_xg[:], scalar2=None,
                                op0=ALU.subtract, op1=ALU.bypass)
                V.tensor_scalar(out=ggs[c][:], in0=ggs[c][:],
                                scalar1=m_g[:], scalar2=None,
                                op0=ALU.subtract, op1=ALU.bypass)

            Mp = []
            with tc.tile_pool(name="gxT", bufs=1) as gxp:
                # ---- xcT, gcT via PE transposes ----
                xcT, gcT = [], []
                for kt in range(KT):
                    tx = gxp.tile([128, C], F32, name=f"xcT{kt}")
                    tg = gxp.tile([128, C], F32, name=f"gcT{kt}")
                    for ci_t in range(CT):
                        tp = sm_psum.tile([128, 128], F32, name="smp")
                        T.transpose(tp[:],
                                    xgs[ci_t][:, kt * 128:(kt + 1) * 128],
                                    ident[:])
                        V.tensor_copy(
                            out=tx[:, ci_t * 128:(ci_t + 1) * 128],
                            in_=tp[:])
                        tp2 = sm_psum.tile([128, 128], F32, name="smp")
                        T.transpose(tp2[:],
                                    ggs[ci_t][:, kt * 128:(kt + 1) * 128],
                                    ident[:])
                        V.tensor_copy(
                            out=tg[:, ci_t * 128:(ci_t + 1) * 128],
                            in_=tp2[:])
                    xcT.append(tx)
                    gcT.append(tg)

                # ---- MT[c',c] = sum_k gc[c',k] xc[c,k] ----
                MT = []
                for cpt in range(CT):
                    ps = mm_psum.tile([128, W], F32, name="mm_ps")
                    for kt in range(KT):
                        T.matmul(ps[:, 0:C],
                                 lhsT=gcT[kt][:, cpt * 128:(cpt + 1) * 128],
                                 rhs=xcT[kt][:, 0:C],
                                 start=(kt == 0), stop=(kt == KT - 1))
                    t = pme.tile([128, C], F32, name=f"MT{cpt}")
                    V.tensor_copy(out=t[:], in_=ps[:, 0:C])
                    MT.append(t)

                # ---- Mp[c,j] = sum_c' MT[c',c] pk[c',j] ----
                for ct in range(CT):
                    ps = mm_psum.tile([128, W], F32, name="mm_ps")
                    for jc in range(2):
                        for cpt in range(CT):
                            T.matmul(ps[:, jc * 512:(jc + 1) * 512],
                                     lhsT=MT[cpt][:, ct * 128:(ct + 1) * 128],
                                     rhs=pk[cpt][:, jc * 512:(jc + 1) * 512],
                                     start=(cpt == 0), stop=(cpt == CT - 1))
                    t = pme.tile([128, W], F32, name=f"Mp{ct}")
                    V.tensor_copy(out=t[:], in_=ps[:])
                    Mp.append(t)

            # ---- energy -> softmax -> att^T ----
            attT = [attp.tile([128, W], F32, name=f"attT_{kt}")
                    for kt in range(KT)]
            for it in range(KT):
                pqblk = []
                for ct in range(CT):
                    blk = strm.tile([128, 128], F32, name=f"pqb{ct}")
                    nc.sync.dma_start(
                        out=blk[:],
                        in_=dram["pq_d"].ap()[ct * 128:(ct + 1) * 128,
                                              it * 128:(it + 1) * 128])
                    pqblk.append(blk)
                ps = mm_psum.tile([128, W], F32, name="mm_ps")
                for jc in range(2):
                    for ct in range(CT):
                        T.matmul(ps[:, jc * 512:(jc + 1) * 512],
                                 lhsT=pqblk[ct][:],
                                 rhs=Mp[ct][:, jc * 512:(jc + 1) * 512],
                                 start=(ct == 0), stop=(ct == CT - 1))
                rowmax = nrm.tile([128, 1], F32, name="rowmax")
                V.tensor_reduce(out=rowmax[:], in_=ps[:], axis=AX.X,
                                op=ALU.max)
                nb = nrm.tile([128, 1], F32, name="negb")
                V.tensor_tensor(out=nb[:], in0=rowmax[:], in1=negalpha[:],
                                op=ALU.mult)
                e = outbuf.tile([128, W], F32, name="ob")
                rowsum = nrm.tile([128, 1], F32, name="rowsum")
                S.activation(e[:], ps[:], AF.Exp, bias=nb[:], scale=alpha[:],
                             accum_out=rowsum[:])
                rs = nrm.tile([128, 1], F32, name="rs")
                V.reciprocal(out=rs[:], in_=rowsum[:])
                V.tensor_scalar_mul(out=e[:], in0=e[:], scalar1=rs[:])
                if DEBUG_DUMP and s == 0:
                    en = outbuf.tile([128, W], F32, name="ob")
                    V.tensor_copy(out=en[:], in_=ps[:])
                    nc.sync.dma_start(
                        out=dram["dbg_energy"].ap()
                        [it * 128:(it + 1) * 128, :], in_=en[:])
                    nc.sync.dma_start(
                        out=dram["dbg_att"].ap()[it * 128:(it + 1) * 128, :],
                        in_=e[:])
                for kt in range(KT):
                    tp = sm_psum.tile([128, 128], F32, name="smp")
                    T.transpose(tp[:], e[:, kt * 128:(kt + 1) * 128],
                                ident[:])
                    V.tensor_copy(out=attT[kt][:, it * 128:(it + 1) * 128],
                                  in_=tp[:])

            if DEBUG_DUMP and s == 0:
                for kt_ in range(KT):
                    nc.sync.dma_start(
                        out=dram["dbg_attT"].ap()
                        [kt_ * 128:(kt_ + 1) * 128, :], in_=attT[kt_][:])

            # ---- out[c,j] = sum_k pv[c,k] att[j,k] ----
            for ct in range(CT):
                ps = mm_psum.tile([128, W], F32, name="mm_ps")
                for jc in range(2):
                    for kt in range(KT):
                        T.matmul(ps[:, jc * 512:(jc + 1) * 512],
                                 lhsT=pvT[kt][:, ct * 128:(ct + 1) * 128],
                                 rhs=attT[kt][:, jc * 512:(jc + 1) * 512],
                                 start=(kt == 0), stop=(kt == KT - 1))
                t = outbuf.tile([128, W], F32, name="ob")
                V.tensor_copy(out=t[:], in_=ps[:])
                nc.sync.dma_start(
                    out=dram["y"].ap()[s, ct * 128:(ct + 1) * 128, :],
                    in_=t[:])


def _build():
    nc = bass.Bass("TRN2", target_bir_lowering=False, debug=False,
                   num_devices=N_CORES)
    dram = {}
    dram["x"] = nc.dram_tensor("x", [SPC, C, W], F32, kind="ExternalInput")
    for nm, shp in [("qw", [C, C, 1]), ("kw", [C, C, 1]), ("vw", [C, C, 1]),
                    ("rw1", [C, C, 3]), ("rw2", [C, C, 3])]:
        dram[nm] = nc.dram_tensor(nm, shp, F32, kind="ExternalInput")
    for nm in ["qb", "kb", "vb", "rb1", "rb2"]:
        dram[nm] = nc.dram_tensor(nm, [C], F32, kind="ExternalInput")
    dram["y"] = nc.dram_tensor("y", [SPC, C, W], F32, kind="ExternalOutput")
    dram["xg_d"] = nc.dram_tensor("xg_d", [SPC, C, W], F32)
    dram["pq_d"] = nc.dram_tensor("pq_d", [C, W], F32)
    dram["g_d"] = nc.dram_tensor("g_d", [SPC, C, W], F32)
    if DEBUG_DUMP:
        dram["dbg_energy"] = nc.dram_tensor("dbg_energy", [W, W], F32,
                                            kind="ExternalOutput")
        dram["dbg_att"] = nc.dram_tensor("dbg_att", [W, W], F32,
                                         kind="ExternalOutput")
        dram["dbg_attT"] = nc.dram_tensor("dbg_attT", [W, W], F32,
                                          kind="ExternalOutput")
    dram["cc_in"] = nc.dram_tensor("cc_in", [1, 4], F32)
    dram["cc_out"] = nc.dram_tensor("cc_out", [1, 4], F32,
                                    addr_space="Shared")

    with tile.TileContext(nc) as tc:
        with ExitStack() as ctx:
            _emit(nc, tc, ctx, dram)
    _split_multiwait(nc)
    return nc


_NC_CACHE = {}


def kernel(**inputs):
    if "nc" not in _NC_CACHE:
        _NC_CACHE["nc"] = _build()
    nc = _NC_CACHE["nc"]
    x = np.ascontiguousarray(np.asarray(inputs["x"], dtype=np.float32))
    common = {}
    for nm in ["qw", "kw", "vw", "rw1", "rw2", "qb", "kb", "vb",
               "rb1", "rb2"]:
        common[nm] = np.ascontiguousarray(
            np.asarray(inputs[nm], dtype=np.float32))
    in_maps = []
    for core in range(N_CORES):
        m = dict(common)
        m["x"] = np.ascontiguousarray(x[core * SPC:(core + 1) * SPC])
        in_maps.append(m)
    res = run_bass_kernel_spmd(nc, in_maps, core_ids=list(range(N_CORES)))
    y = np.concatenate([r["y"] for r in res.results], axis=0)
    return y


# revision 21
# speedup vs baseline: 1.3539x; 1.1053x over previous
"""Trainium2 Bass kernel for nn_CCAR_11579231830663 (dense_transformer).

Data-parallel over batch: 16 samples -> 8 NeuronCores x 2 samples. The global
z-score mean/std of x_g and g are the only cross-core terms; each core
all-reduces 4 scalar sums (sum/sumsq of x_g and g) on-device.

Per sample (C=512, W=1024):
  g   = sin(IN(conv3(x, rw1)+rb1)); g = sin(IN(conv3(g, rw2)+rb2))
  x_g = x + g
  qe  = zscore(x_g)^T . pq,  pq = qw@x_g + qb     (z-score over ALL of x_g)
  ke  = zscore(g)^T  . pk,  pk = kw@g + kb
  energy = qe @ ke ; att = softmax(energy); out = (vw@g+vb) @ att^T
The z-score is folded out: center x_g/g by the global means, compute raw
bilinear forms M1 = xc^T@pq (lhsT for energy), M2 = gc^T@pk (rhs), and fold
alpha = 1/(s_xg*s_g) into the exp: softmax(a*E) = exp(a*(E-rowmax))/sum, a>0.
"""
import sys
sys.path.insert(0, '/opt/trn_rl_repo')

import numpy as np
from contextlib import ExitStack

import concourse.bass as bass
import concourse.tile as tile
from concourse import mybir
from concourse.masks import make_identity
from concourse.bass_utils import run_bass_kernel_spmd

F32 = mybir.dt.float32
AF = mybir.ActivationFunctionType
ALU = mybir.AluOpType
AX = mybir.AxisListType

N_CORES = 8
B, C, W = 16, 512, 1024
SPC = B // N_CORES      # samples per core
CT = C // 128           # channel tiles
KT = W // 128           # width 128-tiles
EPS = 1e-5
NTOT = float(B * C * W)

DEBUG_DUMP = False
OUT_F32R = True
F32R = mybir.dt.float32r
MAGIC = 12582912.0       # 1.5*2^23 fp32 round-to-nearest-int magic
TWOPI = float(2 * np.pi)
INV2PI = float(1.0 / (2 * np.pi))

# ---------------------------------------------------------------------------
# walrus workaround: this container's walrus accepts only a limited number of
# sync waits per instruction; Tile can aggregate more (e.g. the tail drain).
# Split excess waits onto same-engine NOPs placed just before the instruction.
_uid = [0]


def _split_multiwait(nc, limit=1):
    for f in nc.m.functions:
        for bb in f.blocks:
            insts = list(bb.instructions)
            out = []
            changed = False
            for inst in insts:
                si = inst.sync_info
                waits = list(si.on_wait) if si is not None and si.on_wait else []
                if len(waits) > limit:
                    changed = True
                    excess, keep = waits[:-limit], waits[-limit:]
                    si.on_wait = keep
                    inst.sync_info = si
                    for i in range(0, len(excess), limit):
                        chunk = excess[i:i + limit]
                        _uid[0] += 1
                        nop = mybir.InstNoOp(
                            name=f"I-waitsplit-{_uid[0]}", ins=[], outs=[])
                        nop.engine = inst.engine
                        nop.sync_info = mybir.SyncInfo(
                            on_wait=chunk, on_update=[])
                        out.append(nop)
                out.append(inst)
            if changed:
                bb.instructions = out


# ---------------------------------------------------------------------------
def _emit(nc, tc, ctx, dram):
    V = nc.vector
    S = nc.scalar
    T = nc.tensor

    # ---------------- outer pools (small / long-lived) --------------------
    singles = ctx.enter_context(tc.tile_pool(name="singles", bufs=1))
    spool = ctx.enter_context(tc.tile_pool(name="spool", bufs=1))
    nrm = ctx.enter_context(tc.tile_pool(name="nrm", bufs=2))
    outbuf = ctx.enter_context(tc.tile_pool(name="outbuf", bufs=2))
    qkv_w = ctx.enter_context(tc.tile_pool(name="qkv_w", bufs=1))
    big = ctx.enter_context(tc.tile_pool(name="big", bufs=1))
    mm_psum = ctx.enter_context(
        tc.tile_pool(name="mm_psum", bufs=3, space="PSUM"))
    sm_psum = ctx.enter_context(
        tc.tile_pool(name="sm_psum", bufs=2, space="PSUM"))

    ident = singles.tile([128, 128], F32, name="ident")
    make_identity(nc, ident[:])
    identr = singles.tile([128, 128], F32R, name="identr")
    V.tensor_copy(out=identr[:], in_=ident[:])

    ones1 = singles.tile([1, 128], F32, name="ones1")
    V.memset(ones1[:], 1.0)
    ones128 = singles.tile([128, 1], F32, name="ones128")
    V.memset(ones128[:], 1.0)

    def load_bias_cols(name):
        t = singles.tile([128, CT], F32, name=f"{name}_cols")
        src = dram[name].ap().rearrange("(t p) -> p t", p=128)
        nc.sync.dma_start(out=t[:], in_=src)
        return t

    rb1b = load_bias_cols("rb1")
    rb2b = load_bias_cols("rb2")
    qbb = load_bias_cols("qb")
    kbb = load_bias_cols("kb")
    vb_row = singles.tile([1, C], F32, name="vb_row")
    nc.sync.dma_start(out=vb_row[:], in_=dram["vb"].ap()[None, :])
    qb_row = singles.tile([1, C], F32, name="qb_row")
    nc.sync.dma_start(out=qb_row[:], in_=dram["qb"].ap()[None, :])

    # columns: 0 sum_xg, 1 sumsq_xg, 2 sum_g, 3 sumsq_g
    stats_block = spool.tile([128, 4], F32, name="stats_block")
    V.memset(stats_block[:], 0.0)

    # x_g and g live in DRAM between the R phase and the per-sample
    # P/M/E pass; inside each phase they occupy tag-shared SBUF tiles.
    def big_tiles(prefix):
        return [big.tile([128, W], F32, name=f"{prefix}_{c}")
                for c in range(CT)]

    # ---------------- helpers ---------------------------------------------
    def rstd_from_var(varcol):
        """[128,1] biased var -> 1/sqrt(var+eps), Newton-refined."""
        veps = nrm.tile([128, 1], F32, name="veps")
        V.tensor_scalar_add(out=veps[:], in0=varcol, scalar1=EPS)
        s0 = nrm.tile([128, 1], F32, name="s0")
        S.activation(s0[:], veps[:], AF.Sqrt)
        y0 = nrm.tile([128, 1], F32, name="y0")
        V.reciprocal(out=y0[:], in_=s0[:])
        t1 = nrm.tile([128, 1], F32, name="nt1")
        V.tensor_tensor(out=t1[:], in0=y0[:], in1=y0[:], op=ALU.mult)
        V.tensor_tensor(out=t1[:], in0=t1[:], in1=veps[:], op=ALU.mult)
        V.tensor_scalar(out=t1[:], in0=t1[:], scalar1=-0.5, scalar2=1.5,
                        op0=ALU.mult, op1=ALU.add)
        y1 = nrm.tile([128, 1], F32, name="ny1")
        V.tensor_tensor(out=y1[:], in0=y0[:], in1=t1[:], op=ALU.mult)
        return y1

    # ======================= R phase: residual block =======================
    with ExitStack() as rctx:
        conv_w = rctx.enter_context(tc.tile_pool(name="conv_w", bufs=1))
        natp = rctx.enter_context(tc.tile_pool(name="wnat", bufs=1))
        padp = rctx.enter_context(tc.tile_pool(name="padp", bufs=4))
        xre = rctx.enter_context(tc.tile_pool(name="xre", bufs=2))
        scr = rctx.enter_context(tc.tile_pool(name="scr", bufs=2))

        # conv weights rw[cout, cin, k] -> rwT[cin_p, k, cin_t, cout_t, :]
        def load_conv_weightT(name):
            wT = conv_w.tile([128, 3, CT, CT, 128], F32, name=f"{name}T")
            for co_t in range(CT):
                nat = natp.tile([128, C * 3], F32, name="wnat")
                nc.sync.dma_start(
                    out=nat[:],
                    in_=dram[name].ap().rearrange("a b c -> a (b c)")
                    [co_t * 128:(co_t + 1) * 128])
                for k in range(3):
                    for ci_t in range(CT):
                        ps = sm_psum.tile([128, 128], F32, name="smp")
                        src = nat[:, ci_t * 384 + k: (ci_t + 1) * 384: 3]
                        T.transpose(ps[:], src, ident[:])
                        V.tensor_copy(out=wT[:, k, ci_t, co_t, :], in_=ps[:])
            return wT

        rw1T = load_conv_weightT("rw1")
        rw2T = load_conv_weightT("rw2")

        # 1x1 weights [cout, cin, 1] -> wT[cin_p, cin_t, cout]
        def load_1x1_weightT(name):
            wT = qkv_w.tile([128, CT, C], F32, name=f"{name}T")
            for co_t in range(CT):
                nat = natp.tile([128, C * 3], F32, name="wnat")
                nc.sync.dma_start(
                    out=nat[:, 0:C],
                    in_=dram[name].ap().rearrange("a b c -> a (b c)")
                    [co_t * 128:(co_t + 1) * 128])
                for ci_t in range(CT):
                    ps = sm_psum.tile([128, 128], F32, name="smp")
                    T.transpose(ps[:], nat[:, ci_t * 128:(ci_t + 1) * 128],
                                ident[:])
                    V.tensor_copy(
                        out=wT[:, ci_t, co_t * 128:(co_t + 1) * 128],
                        in_=ps[:])
            return wT

        qwT = load_1x1_weightT("qw")
        kwT = load_1x1_weightT("kw")
        vwT = load_1x1_weightT("vw")

        def conv3(dst_cb, src_tiles, wT):
            """3-tap conv from padded [128, W+2] src tiles; dst_cb(co_t, ps)
            gets the accumulated [128, W] PSUM (bias not applied)."""
            for co_t in range(CT):
                ps = mm_psum.tile([128, W], F32, name="mm_ps")
                for jc in range(2):
                    idx = 0
                    for k in range(3):
                        for ci_t in range(CT):
                            T.matmul(ps[:, jc * 512:(jc + 1) * 512],
                                     lhsT=wT[:, k, ci_t, co_t, :],
                                     rhs=src_tiles[ci_t][:, jc * 512 + k:
                                                         jc * 512 + k + 512],
                                     start=(idx == 0), stop=(idx == 11))
                            idx += 1
                dst_cb(co_t, ps)

        def inorm_sin(dst, srcp, bias_col, out_accum=None):
            """dst <- sin(instance_norm(srcp + bias)); srcp: [128, W] PSUM."""
            t = scr.tile([128, W], F32, name="scrA")
            S.activation(t[:], srcp[:], AF.Identity, bias=bias_col)
            st = nrm.tile([128, 2, 6], F32, name="bn_st")
            V.bn_stats(st[:, 0, :], t[:, 0:512])
            V.bn_stats(st[:, 1, :], t[:, 512:1024])
            mv = nrm.tile([128, 2], F32, name="bn_mv")
            V.bn_aggr(mv[:], st[:])
            rstd = rstd_from_var(mv[:, 1:2])
            w = scr.tile([128, W], F32, name="scrB")
            V.tensor_scalar(out=w[:], in0=t[:], scalar1=mv[:, 0:1],
                            scalar2=rstd[:], op0=ALU.subtract, op1=ALU.mult)
            u = scr.tile([128, W], F32, name="scrA")
            V.tensor_scalar(out=u[:], in0=w[:], scalar1=INV2PI, scalar2=MAGIC,
                            op0=ALU.mult, op1=ALU.add)
            V.tensor_scalar(out=u[:], in0=u[:], scalar1=MAGIC, scalar2=None,
                            op0=ALU.subtract, op1=ALU.bypass)
            V.scalar_tensor_tensor(out=u[:], in0=u[:], scalar=-TWOPI,
                                   in1=w[:], op0=ALU.mult, op1=ALU.add)
            S.activation(dst, u[:], AF.Sin, accum_out=out_accum)

        for s in range(SPC):
            xp = [padp.tile([128, W + 2], F32, name="pad") for _ in range(CT)]
            for c in range(CT):
                V.memset(xp[c][:], 0.0)
                nc.sync.dma_start(
                    out=xp[c][:, 1:W + 1],
                    in_=dram["x"].ap()[s, c * 128:(c + 1) * 128, :])

            g1p = [padp.tile([128, W + 2], F32, name="pad") for _ in range(CT)]
            for c in range(CT):
                V.memset(g1p[c][:], 0.0)

            def c1_cb(co_t, ps):
                inorm_sin(g1p[co_t][:, 1:W + 1], ps, rb1b[:, co_t:co_t + 1])

            conv3(c1_cb, xp, rw1T)

            gloc = big_tiles("g")
            gsum = [None] * CT

            def c2_cb(co_t, ps):
                gsum[co_t] = nrm.tile([128, 1], F32, name=f"gsum{co_t}")
                inorm_sin(gloc[co_t][:], ps, rb2b[:, co_t:co_t + 1],
                          out_accum=gsum[co_t][:])

            conv3(c2_cb, g1p, rw2T)

            # x_g = x + g (x re-streamed, in place in the stream tile),
            # sum/sumsq stats, then spill x_g and g to DRAM
            for c in range(CT):
                xt = xre.tile([128, W], F32, name="xre")
                nc.sync.dma_start(
                    out=xt[:],
                    in_=dram["x"].ap()[s, c * 128:(c + 1) * 128, :])
                xs1 = nrm.tile([128, 1], F32, name="xs1")
                V.scalar_tensor_tensor(out=xt[:], in0=xt[:],
                                       scalar=0.0, in1=gloc[c][:],
                                       op0=ALU.add, op1=ALU.add,
                                       accum_out=xs1[:])
                xs2 = nrm.tile([128, 1], F32, name="xs2")
                sq = scr.tile([128, W], F32, name="scrB")
                V.scalar_tensor_tensor(out=sq[:], in0=xt[:], scalar=0.0,
                                       in1=xt[:], op0=ALU.add, op1=ALU.mult,
                                       accum_out=xs2[:])
                gs2 = nrm.tile([128, 1], F32, name="gs2")
                sq2 = scr.tile([128, W], F32, name="scrB")
                V.scalar_tensor_tensor(out=sq2[:], in0=gloc[c][:], scalar=0.0,
                                       in1=gloc[c][:], op0=ALU.add,
                                       op1=ALU.mult, accum_out=gs2[:])
                nc.sync.dma_start(
                    out=dram["xg_d"].ap()[s, c * 128:(c + 1) * 128, :],
                    in_=xt[:])
                nc.sync.dma_start(
                    out=dram["g_d"].ap()[s, c * 128:(c + 1) * 128, :],
                    in_=gloc[c][:])
                V.tensor_tensor(out=stats_block[:, 0:1],
                                in0=stats_block[:, 0:1], in1=xs1[:],
                                op=ALU.add)
                V.tensor_tensor(out=stats_block[:, 1:2],
                                in0=stats_block[:, 1:2], in1=xs2[:],
                                op=ALU.add)
                V.tensor_tensor(out=stats_block[:, 2:3],
                                in0=stats_block[:, 2:3], in1=gsum[c][:],
                                op=ALU.add)
                V.tensor_tensor(out=stats_block[:, 3:4],
                                in0=stats_block[:, 3:4], in1=gs2[:],
                                op=ALU.add)

    # ====================== AllReduce of the 4 sums ========================
    ps4 = sm_psum.tile([128, 128], F32, name="smp")
    T.matmul(ps4[:1, 0:4], lhsT=ones128[:], rhs=stats_block[:],
             start=True, stop=True)
    cc_sb = spool.tile([1, 4], F32, name="cc_sb")
    V.tensor_copy(out=cc_sb[:], in_=ps4[:1, 0:4])
    nc.sync.dma_start(out=dram["cc_in"].ap(), in_=cc_sb[:])
    nc.gpsimd.collective_compute(
        "AllReduce", ALU.add,
        replica_groups=[list(range(N_CORES))],
        ins=[dram["cc_in"].ap()],
        outs=[dram["cc_out"].ap()],
    )
    gstat = spool.tile([128, 4], F32, name="gstat")
    bcast = bass.AP(tensor=dram["cc_out"], offset=0, ap=[[0, 128], [1, 4]])
    nc.sync.dma_start(out=gstat[:], in_=bcast)

    # m = S1/N ; var = (S2 - S1^2/N)/(N-1) ; rs = 1/sqrt(var) (Newton)
    def mean_rs(s1col, s2col, tag):
        m = spool.tile([128, 1], F32, name=f"m_{tag}")
        V.tensor_scalar_mul(out=m[:], in0=s1col, scalar1=1.0 / NTOT)
        t = spool.tile([128, 1], F32, name=f"v_{tag}")
        V.tensor_tensor(out=t[:], in0=s1col, in1=m[:], op=ALU.mult)
        V.tensor_scalar_mul(out=t[:], in0=t[:], scalar1=-1.0)
        V.tensor_tensor(out=t[:], in0=t[:], in1=s2col, op=ALU.add)
        V.tensor_scalar_mul(out=t[:], in0=t[:], scalar1=1.0 / (NTOT - 1.0))
        sq = spool.tile([128, 1], F32, name=f"sq_{tag}")
        S.activation(sq[:], t[:], AF.Sqrt)
        y0 = spool.tile([128, 1], F32, name=f"y0_{tag}")
        V.reciprocal(out=y0[:], in_=sq[:])
        t2 = spool.tile([128, 1], F32, name=f"t2_{tag}")
        V.tensor_tensor(out=t2[:], in0=y0[:], in1=y0[:], op=ALU.mult)
        V.tensor_tensor(out=t2[:], in0=t2[:], in1=t[:], op=ALU.mult)
        V.tensor_scalar(out=t2[:], in0=t2[:], scalar1=-0.5, scalar2=1.5,
                        op0=ALU.mult, op1=ALU.add)
        V.tensor_tensor(out=t2[:], in0=y0[:], in1=t2[:], op=ALU.mult)
        return m, t2

    m_xg, rs_xg = mean_rs(gstat[:, 0:1], gstat[:, 1:2], "xg")
    m_g, rs_g = mean_rs(gstat[:, 2:3], gstat[:, 3:4], "g")
    alpha = spool.tile([128, 1], F32, name="alpha")
    V.tensor_tensor(out=alpha[:], in0=rs_xg[:], in1=rs_g[:], op=ALU.mult)
    negalpha = spool.tile([128, 1], F32, name="negalpha")
    V.tensor_scalar_mul(out=negalpha[:], in0=alpha[:], scalar1=-1.0)

    # =================== P/E phases, one sample at a time ==================
    # Reference algebra: energy = a * pq^T @ (xc @ gc^T) @ pk, a = 1/(sx*sg)
    #   MT[c',c] = sum_k gc[c',k] xc[c,k]      (via gcT, xcT: k-partitioned)
    #   Mp[c,j]  = sum_c' MT[c',c] pk[c',j]
    #   E[i,j]   = sum_c pq[c,i] Mp[c,j]
    with ExitStack() as ectx:
        pme = ectx.enter_context(tc.tile_pool(name="pme", bufs=1))
        attp = ectx.enter_context(tc.tile_pool(name="attp", bufs=1))
        strm = ectx.enter_context(tc.tile_pool(name="strm", bufs=2))

        def projT(src_tiles, wT, b_row, prefix):
            """proj^T[k, c] = sum_ci src[ci, k] w[c, ci] + b[c]"""
            res = []
            for kt in range(KT):
                ps = mm_psum.tile([128, W], F32, name="mm_ps")
                for ci_t in range(CT):
                    T.matmul(ps[:, 0:C],
                             lhsT=src_tiles[ci_t][:, kt * 128:(kt + 1) * 128],
                             rhs=wT[:, ci_t, :],
                             start=(ci_t == 0), stop=False)
                T.matmul(ps[:, 0:C], lhsT=ones1[:], rhs=b_row[:],
                         start=False, stop=True)
                t = pme.tile([128, C], F32R if OUT_F32R else F32,
                             name=f"{prefix}{kt}")
                V.tensor_copy(out=t[:], in_=ps[:, 0:C])
                res.append(t)
            return res

        def proj_col(src_tiles, wT, bias_cols, prefix, spill=None):
            """proj[c, j] natural column form; optionally DMA to DRAM
            (spilled projections use rotating outbuf slots)."""
            res = []
            for co_t in range(CT):
                ps = mm_psum.tile([128, W], F32, name="mm_ps")
                for jc in range(2):
                    for ci_t in range(CT):
                        T.matmul(
                            ps[:, jc * 512:(jc + 1) * 512],
                            lhsT=wT[:, ci_t, co_t * 128:(co_t + 1) * 128],
                            rhs=src_tiles[ci_t][:, jc * 512:(jc + 1) * 512],
                            start=(ci_t == 0), stop=(ci_t == CT - 1))
                if spill is not None:
                    t = outbuf.tile([128, W], F32, name="ob")
                else:
                    t = pme.tile([128, W], F32, name=f"{prefix}{co_t}")
                S.activation(t[:], ps[:], AF.Identity,
                             bias=bias_cols[:, co_t:co_t + 1])
                if spill is not None:
                    nc.sync.dma_start(
                        out=dram[spill].ap()[co_t * 128:(co_t + 1) * 128, :],
                        in_=t[:])
                res.append(t)
            return res

        for s in range(SPC):
            # ---- reload x_g, g ----
            xgs = big_tiles("xg")
            ggs = big_tiles("g")
            for c in range(CT):
                nc.sync.dma_start(
                    out=xgs[c][:],
                    in_=dram["xg_d"].ap()[s, c * 128:(c + 1) * 128, :])
                nc.sync.dma_start(
                    out=ggs[c][:],
                    in_=dram["g_d"].ap()[s, c * 128:(c + 1) * 128, :])

            # ---- projections on raw x_g / g; pq spilled to DRAM ----
            proj_col(xgs, qwT, qbb, "pj", spill="pq_d")
            pk = proj_col(ggs, kwT, kbb, "pk")
            pvT = projT(ggs, vwT, vb_row, "pvT")

            # ---- center in place (needs the AllReduce result) ----
            for c in range(CT):
                V.tensor_scalar(out=xgs[c][:], in0=xgs[c][:],
                                scalar1=m_xg[:], scalar2=None,
                                op0=ALU.subtract, op1=ALU.bypass)
                V.tensor_scalar(out=ggs[c][:], in0=ggs[c][:],
                                scalar1=m_g[:], scalar2=None,
                                op0=ALU.subtract, op1=ALU.bypass)

            Mp = []
            with tc.tile_pool(name="gxT", bufs=1) as gxp:
                # ---- xcT, gcT via PE transposes ----
                xcT, gcT = [], []
                for kt in range(KT):
                    tx = gxp.tile([128, C], F32, name=f"xcT{kt}")
                    tg = gxp.tile([128, C], F32, name=f"gcT{kt}")
                    for ci_t in range(CT):
                        tp = sm_psum.tile([128, 128], F32, name="smp")
                        T.transpose(tp[:],
                                    xgs[ci_t][:, kt * 128:(kt + 1) * 128],
                                    ident[:])
                        V.tensor_copy(
                            out=tx[:, ci_t * 128:(ci_t + 1) * 128],
                            in_=tp[:])
                        tp2 = sm_psum.tile([128, 128], F32, name="smp")
                        T.transpose(tp2[:],
                                    ggs[ci_t][:, kt * 128:(kt + 1) * 128],
                                    ident[:])
                        V.tensor_copy(
                            out=tg[:, ci_t * 128:(ci_t + 1) * 128],
                            in_=tp2[:])
                    xcT.append(tx)
                    gcT.append(tg)

                # ---- MT[c',c] = sum_k gc[c',k] xc[c,k] ----
                MT = []
                for cpt in range(CT):
                    ps = mm_psum.tile([128, W], F32, name="mm_ps")
                    for kt in range(KT):
                        T.matmul(ps[:, 0:C],
                                 lhsT=gcT[kt][:, cpt * 128:(cpt + 1) * 128],
                                 rhs=xcT[kt][:, 0:C],
                                 start=(kt == 0), stop=(kt == KT - 1))
                    t = pme.tile([128, C], F32, name=f"MT{cpt}")
                    V.tensor_copy(out=t[:], in_=ps[:, 0:C])
                    MT.append(t)

                # ---- Mp[c,j] = sum_c' MT[c',c] pk[c',j] ----
                for ct in range(CT):
                    ps = mm_psum.tile([128, W], F32, name="mm_ps")
                    for jc in range(2):
                        for cpt in range(CT):
                            T.matmul(ps[:, jc * 512:(jc + 1) * 512],
                                     lhsT=MT[cpt][:, ct * 128:(ct + 1) * 128],
                                     rhs=pk[cpt][:, jc * 512:(jc + 1) * 512],
                                     start=(cpt == 0), stop=(cpt == CT - 1))
                    t = pme.tile([128, W], F32, name=f"Mp{ct}")
                    V.tensor_copy(out=t[:], in_=ps[:])
                    Mp.append(t)

            # ---- energy -> softmax -> att^T ----
            attT = [attp.tile([128, W], F32R if OUT_F32R else F32,
                              name=f"attT_{kt}")
                    for kt in range(KT)]
            for it in range(KT):
                pqblk = []
                for ct in range(CT):
                    blk = strm.tile([128, 128], F32, name=f"pqb{ct}")
                    nc.sync.dma_start(
                        out=blk[:],
                        in_=dram["pq_d"].ap()[ct * 128:(ct + 1) * 128,
                                              it * 128:(it + 1) * 128])
                    pqblk.append(blk)
                ps = mm_psum.tile([128, W], F32, name="mm_ps")
                for jc in range(2):
                    for ct in range(CT):
                        T.matmul(ps[:, jc * 512:(jc + 1) * 512],
                                 lhsT=pqblk[ct][:],
                                 rhs=Mp[ct][:, jc * 512:(jc + 1) * 512],
                                 start=(ct == 0), stop=(ct == CT - 1))
                rowmax = nrm.tile([128, 1], F32, name="rowmax")
                V.tensor_reduce(out=rowmax[:], in_=ps[:], axis=AX.X,
                                op=ALU.max)
                nb = nrm.tile([128, 1], F32, name="negb")
                V.tensor_tensor(out=nb[:], in0=rowmax[:], in1=negalpha[:],
                                op=ALU.mult)
                e = outbuf.tile([128, W], F32, name="ob")
                rowsum = nrm.tile([128, 1], F32, name="rowsum")
                S.activation(e[:], ps[:], AF.Exp, bias=nb[:], scale=alpha[:],
                             accum_out=rowsum[:])
                rs = nrm.tile([128, 1], F32, name="rs")
                V.reciprocal(out=rs[:], in_=rowsum[:])
                if OUT_F32R:
                    er = outbuf.tile([128, W], F32R, name="obr")
                    V.tensor_scalar_mul(out=er[:], in0=e[:], scalar1=rs[:])
                    e = er
                else:
                    V.tensor_scalar_mul(out=e[:], in0=e[:], scalar1=rs[:])
                if DEBUG_DUMP and s == 0:
                    en = outbuf.tile([128, W], F32, name="ob")
                    V.tensor_copy(out=en[:], in_=ps[:])
                    nc.sync.dma_start(
                        out=dram["dbg_energy"].ap()
                        [it * 128:(it + 1) * 128, :], in_=en[:])
                    nc.sync.dma_start(
                        out=dram["dbg_att"].ap()[it * 128:(it + 1) * 128, :],
                        in_=e[:])
                for kt in range(KT):
                    if OUT_F32R:
                        tp = sm_psum.tile([128, 128], F32R, name="smp")
                        T.transpose(tp[:], e[:, kt * 128:(kt + 1) * 128],
                                    identr[:])
                    else:
                        tp = sm_psum.tile([128, 128], F32, name="smp")
                        T.transpose(tp[:], e[:, kt * 128:(kt + 1) * 128],
                                    ident[:])
                    V.tensor_copy(out=attT[kt][:, it * 128:(it + 1) * 128],
                                  in_=tp[:])

            if DEBUG_DUMP and s == 0:
                for kt_ in range(KT):
                    nc.sync.dma_start(
                        out=dram["dbg_attT"].ap()
                        [kt_ * 128:(kt_ + 1) * 128, :], in_=attT[kt_][:])

            # ---- out[c,j] = sum_k pv[c,k] att[j,k] ----
            for ct in range(CT):
                ps = mm_psum.tile([128, W], F32, name="mm_ps")
                for jc in range(2):
                    for kt in range(KT):
                        T.matmul(ps[:, jc * 512:(jc + 1) * 512],
                                 lhsT=pvT[kt][:, ct * 128:(ct + 1) * 128],
                                 rhs=attT[kt][:, jc * 512:(jc + 1) * 512],
                                 start=(kt == 0), stop=(kt == KT - 1))
                t = outbuf.tile([128, W], F32, name="ob")
                V.tensor_copy(out=t[:], in_=ps[:])
                nc.sync.dma_start(
                    out=dram["y"].ap()[s, ct * 128:(ct + 1) * 128, :],
                    in_=t[:])


def _build():
    nc = bass.Bass("TRN2", target_bir_lowering=False, debug=False,
                   num_devices=N_CORES)
    dram = {}
    dram["x"] = nc.dram_tensor("x", [SPC, C, W], F32, kind="ExternalInput")
    for nm, shp in [("qw", [C, C, 1]), ("kw", [C, C, 1]), ("vw", [C, C, 1]),
                    ("rw1", [C, C, 3]), ("rw2", [C, C, 3])]:
        dram[nm] = nc.dram_tensor(nm, shp, F32, kind="ExternalInput")
    for nm in ["qb", "kb", "vb", "rb1", "rb2"]:
        dram[nm] = nc.dram_tensor(nm, [C], F32, kind="ExternalInput")
    dram["y"] = nc.dram_tensor("y", [SPC, C, W], F32, kind="ExternalOutput")
    dram["xg_d"] = nc.dram_tensor("xg_d", [SPC, C, W], F32)
    dram["pq_d"] = nc.dram_tensor("pq_d", [C, W], F32)
    dram["g_d"] = nc.dram_tensor("g_d", [SPC, C, W], F32)
    if DEBUG_DUMP:
        dram["dbg_energy"] = nc.dram_tensor("dbg_energy", [W, W], F32,
                                            kind="ExternalOutput")
        dram["dbg_att"] = nc.dram_tensor("dbg_att", [W, W], F32,
                                         kind="ExternalOutput")
        dram["dbg_attT"] = nc.dram_tensor("dbg_attT", [W, W], F32,
                                          kind="ExternalOutput")
    dram["cc_in"] = nc.dram_tensor("cc_in", [1, 4], F32)
    dram["cc_out"] = nc.dram_tensor("cc_out", [1, 4], F32,
                                    addr_space="Shared")

    with tile.TileContext(nc) as tc:
        with ExitStack() as ctx:
            _emit(nc, tc, ctx, dram)
    _split_multiwait(nc)
    return nc


_NC_CACHE = {}


def kernel(**inputs):
    if "nc" not in _NC_CACHE:
        _NC_CACHE["nc"] = _build()
    nc = _NC_CACHE["nc"]
    x = np.ascontiguousarray(np.asarray(inputs["x"], dtype=np.float32))
    common = {}
    for nm in ["qw", "kw", "vw", "rw1", "rw2", "qb", "kb", "vb",
               "rb1", "rb2"]:
        common[nm] = np.ascontiguousarray(
            np.asarray(inputs[nm], dtype=np.float32))
    in_maps = []
    for core in range(N_CORES):
        m = dict(common)
        m["x"] = np.ascontiguousarray(x[core * SPC:(core + 1) * SPC])
        in_maps.append(m)
    res = run_bass_kernel_spmd(nc, in_maps, core_ids=list(range(N_CORES)))
    y = np.concatenate([r["y"] for r in res.results], axis=0)
    return y


# revision 22
# speedup vs baseline: 1.4234x; 1.0513x over previous
"""Trainium2 Bass kernel for nn_CCAR_11579231830663 (dense_transformer).

Data-parallel over batch: 16 samples -> 8 NeuronCores x 2 samples. The global
z-score mean/std of x_g and g are the only cross-core terms; each core
all-reduces 4 scalar sums (sum/sumsq of x_g and g) on-device.

Per sample (C=512, W=1024):
  g   = sin(IN(conv3(x, rw1)+rb1)); g = sin(IN(conv3(g, rw2)+rb2))
  x_g = x + g
  qe  = zscore(x_g)^T . pq,  pq = qw@x_g + qb     (z-score over ALL of x_g)
  ke  = zscore(g)^T  . pk,  pk = kw@g + kb
  energy = qe @ ke ; att = softmax(energy); out = (vw@g+vb) @ att^T
The z-score is folded out: center x_g/g by the global means, compute raw
bilinear forms M1 = xc^T@pq (lhsT for energy), M2 = gc^T@pk (rhs), and fold
alpha = 1/(s_xg*s_g) into the exp: softmax(a*E) = exp(a*(E-rowmax))/sum, a>0.
"""
import sys
sys.path.insert(0, '/opt/trn_rl_repo')

import numpy as np
from contextlib import ExitStack

import concourse.bass as bass
import concourse.tile as tile
from concourse import mybir
from concourse.masks import make_identity
from concourse.bass_utils import run_bass_kernel_spmd

F32 = mybir.dt.float32
AF = mybir.ActivationFunctionType
ALU = mybir.AluOpType
AX = mybir.AxisListType

N_CORES = 8
B, C, W = 16, 512, 1024
SPC = B // N_CORES      # samples per core
CT = C // 128           # channel tiles
KT = W // 128           # width 128-tiles
EPS = 1e-5
NTOT = float(B * C * W)

DEBUG_DUMP = False
OUT_F32R = True
F32R = mybir.dt.float32r
MAGIC = 12582912.0       # 1.5*2^23 fp32 round-to-nearest-int magic
TWOPI = float(2 * np.pi)
INV2PI = float(1.0 / (2 * np.pi))

# ---------------------------------------------------------------------------
# walrus workaround: this container's walrus accepts only a limited number of
# sync waits per instruction; Tile can aggregate more (e.g. the tail drain).
# Split excess waits onto same-engine NOPs placed just before the instruction.
_uid = [0]


def _split_multiwait(nc, limit=1):
    for f in nc.m.functions:
        for bb in f.blocks:
            insts = list(bb.instructions)
            out = []
            changed = False
            for inst in insts:
                si = inst.sync_info
                waits = list(si.on_wait) if si is not None and si.on_wait else []
                if len(waits) > limit:
                    changed = True
                    excess, keep = waits[:-limit], waits[-limit:]
                    si.on_wait = keep
                    inst.sync_info = si
                    for i in range(0, len(excess), limit):
                        chunk = excess[i:i + limit]
                        _uid[0] += 1
                        nop = mybir.InstNoOp(
                            name=f"I-waitsplit-{_uid[0]}", ins=[], outs=[])
                        nop.engine = inst.engine
                        nop.sync_info = mybir.SyncInfo(
                            on_wait=chunk, on_update=[])
                        out.append(nop)
                out.append(inst)
            if changed:
                bb.instructions = out


# ---------------------------------------------------------------------------
def _emit(nc, tc, ctx, dram):
    V = nc.vector
    S = nc.scalar
    T = nc.tensor

    # ---------------- outer pools (small / long-lived) --------------------
    singles = ctx.enter_context(tc.tile_pool(name="singles", bufs=1))
    spool = ctx.enter_context(tc.tile_pool(name="spool", bufs=1))
    nrm = ctx.enter_context(tc.tile_pool(name="nrm", bufs=2))
    outbuf = ctx.enter_context(tc.tile_pool(name="outbuf", bufs=2))
    qkv_w = ctx.enter_context(tc.tile_pool(name="qkv_w", bufs=1))
    big = ctx.enter_context(tc.tile_pool(name="big", bufs=1))
    mm_psum = ctx.enter_context(
        tc.tile_pool(name="mm_psum", bufs=3, space="PSUM"))
    sm_psum = ctx.enter_context(
        tc.tile_pool(name="sm_psum", bufs=2, space="PSUM"))

    ident = singles.tile([128, 128], F32, name="ident")
    make_identity(nc, ident[:])
    identr = singles.tile([128, 128], F32R, name="identr")
    V.tensor_copy(out=identr[:], in_=ident[:])

    ones1 = singles.tile([1, 128], F32, name="ones1")
    V.memset(ones1[:], 1.0)
    ones128 = singles.tile([128, 1], F32, name="ones128")
    V.memset(ones128[:], 1.0)

    def load_bias_cols(name):
        t = singles.tile([128, CT], F32, name=f"{name}_cols")
        src = dram[name].ap().rearrange("(t p) -> p t", p=128)
        nc.sync.dma_start(out=t[:], in_=src)
        return t

    rb1b = load_bias_cols("rb1")
    rb2b = load_bias_cols("rb2")
    qbb = load_bias_cols("qb")
    kbb = load_bias_cols("kb")
    vb_bc = singles.tile([128, C], F32, name="vb_bc")
    nc.sync.dma_start(out=vb_bc[:],
                      in_=bass.AP(tensor=dram["vb"], offset=0,
                                  ap=[[0, 128], [1, C]]))

    # columns: 0 sum_xg, 1 sumsq_xg, 2 sum_g, 3 sumsq_g
    stats_block = spool.tile([128, 4], F32, name="stats_block")
    V.memset(stats_block[:], 0.0)

    # x_g and g live in DRAM between the R phase and the per-sample
    # P/M/E pass; inside each phase they occupy tag-shared SBUF tiles.
    def big_tiles(prefix):
        return [big.tile([128, W], F32, name=f"{prefix}_{c}")
                for c in range(CT)]

    # ---------------- helpers ---------------------------------------------
    def rstd_from_var(varcol):
        """[128,1] biased var -> 1/sqrt(var+eps), Newton-refined."""
        veps = nrm.tile([128, 1], F32, name="veps")
        V.tensor_scalar_add(out=veps[:], in0=varcol, scalar1=EPS)
        s0 = nrm.tile([128, 1], F32, name="s0")
        S.activation(s0[:], veps[:], AF.Sqrt)
        y0 = nrm.tile([128, 1], F32, name="y0")
        V.reciprocal(out=y0[:], in_=s0[:])
        t1 = nrm.tile([128, 1], F32, name="nt1")
        V.tensor_tensor(out=t1[:], in0=y0[:], in1=y0[:], op=ALU.mult)
        V.tensor_tensor(out=t1[:], in0=t1[:], in1=veps[:], op=ALU.mult)
        V.tensor_scalar(out=t1[:], in0=t1[:], scalar1=-0.5, scalar2=1.5,
                        op0=ALU.mult, op1=ALU.add)
        y1 = nrm.tile([128, 1], F32, name="ny1")
        V.tensor_tensor(out=y1[:], in0=y0[:], in1=t1[:], op=ALU.mult)
        return y1

    # ======================= R phase: residual block =======================
    with ExitStack() as rctx:
        conv_w = rctx.enter_context(tc.tile_pool(name="conv_w", bufs=1))
        natp = rctx.enter_context(tc.tile_pool(name="wnat", bufs=1))
        padp = rctx.enter_context(tc.tile_pool(name="padp", bufs=4))
        xre = rctx.enter_context(tc.tile_pool(name="xre", bufs=2))
        scr = rctx.enter_context(tc.tile_pool(name="scr", bufs=2))

        # conv weights rw[cout, cin, k] -> rwT[cin_p, k, cin_t, cout_t, :]
        def load_conv_weightT(name):
            wT = conv_w.tile([128, 3, CT, CT, 128], F32, name=f"{name}T")
            for co_t in range(CT):
                nat = natp.tile([128, C * 3], F32, name="wnat")
                nc.sync.dma_start(
                    out=nat[:],
                    in_=dram[name].ap().rearrange("a b c -> a (b c)")
                    [co_t * 128:(co_t + 1) * 128])
                for k in range(3):
                    for ci_t in range(CT):
                        ps = sm_psum.tile([128, 128], F32, name="smp")
                        src = nat[:, ci_t * 384 + k: (ci_t + 1) * 384: 3]
                        T.transpose(ps[:], src, ident[:])
                        V.tensor_copy(out=wT[:, k, ci_t, co_t, :], in_=ps[:])
            return wT

        rw1T = load_conv_weightT("rw1")
        rw2T = load_conv_weightT("rw2")

        # 1x1 weights [cout, cin, 1] -> wT[cin_p, cin_t, cout]
        def load_1x1_weightT(name, dtype=F32):
            wT = qkv_w.tile([128, CT, C], dtype, name=f"{name}T")
            for co_t in range(CT):
                nat = natp.tile([128, C * 3], F32, name="wnat")
                nc.sync.dma_start(
                    out=nat[:, 0:C],
                    in_=dram[name].ap().rearrange("a b c -> a (b c)")
                    [co_t * 128:(co_t + 1) * 128])
                for ci_t in range(CT):
                    ps = sm_psum.tile([128, 128], F32, name="smp")
                    T.transpose(ps[:], nat[:, ci_t * 128:(ci_t + 1) * 128],
                                ident[:])
                    V.tensor_copy(
                        out=wT[:, ci_t, co_t * 128:(co_t + 1) * 128],
                        in_=ps[:])
            return wT

        qwT = load_1x1_weightT("qw")
        kwT = load_1x1_weightT("kw")
        vwT = load_1x1_weightT("vw", dtype=F32R)

        def conv3(dst_cb, src_tiles, wT):
            """3-tap conv from padded [128, W+2] src tiles; dst_cb(co_t, ps)
            gets the accumulated [128, W] PSUM (bias not applied)."""
            for co_t in range(CT):
                ps = mm_psum.tile([128, W], F32, name="mm_ps")
                for jc in range(2):
                    idx = 0
                    for k in range(3):
                        for ci_t in range(CT):
                            T.matmul(ps[:, jc * 512:(jc + 1) * 512],
                                     lhsT=wT[:, k, ci_t, co_t, :],
                                     rhs=src_tiles[ci_t][:, jc * 512 + k:
                                                         jc * 512 + k + 512],
                                     start=(idx == 0), stop=(idx == 11))
                            idx += 1
                dst_cb(co_t, ps)

        def inorm_sin(dst, srcp, bias_col, out_accum=None):
            """dst <- sin(instance_norm(srcp + bias)); srcp: [128, W] PSUM."""
            t = scr.tile([128, W], F32, name="scrA")
            S.activation(t[:], srcp[:], AF.Identity, bias=bias_col)
            st = nrm.tile([128, 2, 6], F32, name="bn_st")
            V.bn_stats(st[:, 0, :], t[:, 0:512])
            V.bn_stats(st[:, 1, :], t[:, 512:1024])
            mv = nrm.tile([128, 2], F32, name="bn_mv")
            V.bn_aggr(mv[:], st[:])
            rstd = rstd_from_var(mv[:, 1:2])
            w = scr.tile([128, W], F32, name="scrB")
            V.tensor_scalar(out=w[:], in0=t[:], scalar1=mv[:, 0:1],
                            scalar2=rstd[:], op0=ALU.subtract, op1=ALU.mult)
            u = scr.tile([128, W], F32, name="scrA")
            V.tensor_scalar(out=u[:], in0=w[:], scalar1=INV2PI, scalar2=MAGIC,
                            op0=ALU.mult, op1=ALU.add)
            V.tensor_scalar(out=u[:], in0=u[:], scalar1=MAGIC, scalar2=None,
                            op0=ALU.subtract, op1=ALU.bypass)
            V.scalar_tensor_tensor(out=u[:], in0=u[:], scalar=-TWOPI,
                                   in1=w[:], op0=ALU.mult, op1=ALU.add)
            S.activation(dst, u[:], AF.Sin, accum_out=out_accum)

        for s in range(SPC):
            xp = [padp.tile([128, W + 2], F32, name="pad") for _ in range(CT)]
            for c in range(CT):
                V.memset(xp[c][:], 0.0)
                nc.sync.dma_start(
                    out=xp[c][:, 1:W + 1],
                    in_=dram["x"].ap()[s, c * 128:(c + 1) * 128, :])

            g1p = [padp.tile([128, W + 2], F32, name="pad") for _ in range(CT)]
            for c in range(CT):
                V.memset(g1p[c][:], 0.0)

            def c1_cb(co_t, ps):
                inorm_sin(g1p[co_t][:, 1:W + 1], ps, rb1b[:, co_t:co_t + 1])

            conv3(c1_cb, xp, rw1T)

            gloc = big_tiles("g")
            gsum = [None] * CT

            def c2_cb(co_t, ps):
                gsum[co_t] = nrm.tile([128, 1], F32, name=f"gsum{co_t}")
                inorm_sin(gloc[co_t][:], ps, rb2b[:, co_t:co_t + 1],
                          out_accum=gsum[co_t][:])

            conv3(c2_cb, g1p, rw2T)

            # x_g = x + g (x re-streamed, in place in the stream tile),
            # sum/sumsq stats, then spill x_g and g to DRAM
            for c in range(CT):
                xt = xre.tile([128, W], F32, name="xre")
                nc.sync.dma_start(
                    out=xt[:],
                    in_=dram["x"].ap()[s, c * 128:(c + 1) * 128, :])
                xs1 = nrm.tile([128, 1], F32, name="xs1")
                V.scalar_tensor_tensor(out=xt[:], in0=xt[:],
                                       scalar=0.0, in1=gloc[c][:],
                                       op0=ALU.add, op1=ALU.add,
                                       accum_out=xs1[:])
                xs2 = nrm.tile([128, 1], F32, name="xs2")
                sq = scr.tile([128, W], F32, name="scrB")
                V.scalar_tensor_tensor(out=sq[:], in0=xt[:], scalar=0.0,
                                       in1=xt[:], op0=ALU.add, op1=ALU.mult,
                                       accum_out=xs2[:])
                gs2 = nrm.tile([128, 1], F32, name="gs2")
                sq2 = scr.tile([128, W], F32, name="scrB")
                V.scalar_tensor_tensor(out=sq2[:], in0=gloc[c][:], scalar=0.0,
                                       in1=gloc[c][:], op0=ALU.add,
                                       op1=ALU.mult, accum_out=gs2[:])
                nc.sync.dma_start(
                    out=dram["xg_d"].ap()[s, c * 128:(c + 1) * 128, :],
                    in_=xt[:])
                nc.sync.dma_start(
                    out=dram["g_d"].ap()[s, c * 128:(c + 1) * 128, :],
                    in_=gloc[c][:])
                V.tensor_tensor(out=stats_block[:, 0:1],
                                in0=stats_block[:, 0:1], in1=xs1[:],
                                op=ALU.add)
                V.tensor_tensor(out=stats_block[:, 1:2],
                                in0=stats_block[:, 1:2], in1=xs2[:],
                                op=ALU.add)
                V.tensor_tensor(out=stats_block[:, 2:3],
                                in0=stats_block[:, 2:3], in1=gsum[c][:],
                                op=ALU.add)
                V.tensor_tensor(out=stats_block[:, 3:4],
                                in0=stats_block[:, 3:4], in1=gs2[:],
                                op=ALU.add)

    # ====================== AllReduce of the 4 sums ========================
    ps4 = sm_psum.tile([128, 128], F32, name="smp")
    T.matmul(ps4[:1, 0:4], lhsT=ones128[:], rhs=stats_block[:],
             start=True, stop=True)
    cc_sb = spool.tile([1, 4], F32, name="cc_sb")
    V.tensor_copy(out=cc_sb[:], in_=ps4[:1, 0:4])
    nc.sync.dma_start(out=dram["cc_in"].ap(), in_=cc_sb[:])
    nc.gpsimd.collective_compute(
        "AllReduce", ALU.add,
        replica_groups=[list(range(N_CORES))],
        ins=[dram["cc_in"].ap()],
        outs=[dram["cc_out"].ap()],
    )
    gstat = spool.tile([128, 4], F32, name="gstat")
    bcast = bass.AP(tensor=dram["cc_out"], offset=0, ap=[[0, 128], [1, 4]])
    nc.sync.dma_start(out=gstat[:], in_=bcast)

    # m = S1/N ; var = (S2 - S1^2/N)/(N-1) ; rs = 1/sqrt(var) (Newton)
    def mean_rs(s1col, s2col, tag):
        m = spool.tile([128, 1], F32, name=f"m_{tag}")
        V.tensor_scalar_mul(out=m[:], in0=s1col, scalar1=1.0 / NTOT)
        t = spool.tile([128, 1], F32, name=f"v_{tag}")
        V.tensor_tensor(out=t[:], in0=s1col, in1=m[:], op=ALU.mult)
        V.tensor_scalar_mul(out=t[:], in0=t[:], scalar1=-1.0)
        V.tensor_tensor(out=t[:], in0=t[:], in1=s2col, op=ALU.add)
        V.tensor_scalar_mul(out=t[:], in0=t[:], scalar1=1.0 / (NTOT - 1.0))
        sq = spool.tile([128, 1], F32, name=f"sq_{tag}")
        S.activation(sq[:], t[:], AF.Sqrt)
        y0 = spool.tile([128, 1], F32, name=f"y0_{tag}")
        V.reciprocal(out=y0[:], in_=sq[:])
        t2 = spool.tile([128, 1], F32, name=f"t2_{tag}")
        V.tensor_tensor(out=t2[:], in0=y0[:], in1=y0[:], op=ALU.mult)
        V.tensor_tensor(out=t2[:], in0=t2[:], in1=t[:], op=ALU.mult)
        V.tensor_scalar(out=t2[:], in0=t2[:], scalar1=-0.5, scalar2=1.5,
                        op0=ALU.mult, op1=ALU.add)
        V.tensor_tensor(out=t2[:], in0=y0[:], in1=t2[:], op=ALU.mult)
        return m, t2

    m_xg, rs_xg = mean_rs(gstat[:, 0:1], gstat[:, 1:2], "xg")
    m_g, rs_g = mean_rs(gstat[:, 2:3], gstat[:, 3:4], "g")
    alpha = spool.tile([128, 1], F32, name="alpha")
    V.tensor_tensor(out=alpha[:], in0=rs_xg[:], in1=rs_g[:], op=ALU.mult)
    negalpha = spool.tile([128, 1], F32, name="negalpha")
    V.tensor_scalar_mul(out=negalpha[:], in0=alpha[:], scalar1=-1.0)

    # =================== P/E phases, one sample at a time ==================
    # Reference algebra: energy = a * pq^T @ (xc @ gc^T) @ pk, a = 1/(sx*sg)
    #   MT[c',c] = sum_k gc[c',k] xc[c,k]      (via gcT, xcT: k-partitioned)
    #   Mp[c,j]  = sum_c' MT[c',c] pk[c',j]
    #   E[i,j]   = sum_c pq[c,i] Mp[c,j]
    with ExitStack() as ectx:
        pme = ectx.enter_context(tc.tile_pool(name="pme", bufs=1))
        attp = ectx.enter_context(tc.tile_pool(name="attp", bufs=1))
        strm = ectx.enter_context(tc.tile_pool(name="strm", bufs=2))

        def projT(src_tiles, wT, b_bc, prefix):
            """proj^T[k, c] = sum_ci src[ci, k] w[c, ci] + b[c] (f32r)"""
            res = []
            for kt in range(KT):
                ps = mm_psum.tile([128, W], F32, name="mm_ps")
                for ci_t in range(CT):
                    T.matmul(ps[:, 0:C],
                             lhsT=src_tiles[ci_t][:, kt * 128:(kt + 1) * 128],
                             rhs=wT[:, ci_t, :],
                             start=(ci_t == 0), stop=(ci_t == CT - 1))
                t = pme.tile([128, C], F32R, name=f"{prefix}{kt}")
                V.scalar_tensor_tensor(out=t[:], in0=ps[:, 0:C], scalar=0.0,
                                       in1=b_bc[:], op0=ALU.add, op1=ALU.add)
                res.append(t)
            return res

        def proj_col(src_tiles, wT, bias_cols, prefix, spill=None):
            """proj[c, j] natural column form; optionally DMA to DRAM
            (spilled projections use rotating outbuf slots)."""
            res = []
            for co_t in range(CT):
                ps = mm_psum.tile([128, W], F32, name="mm_ps")
                for jc in range(2):
                    for ci_t in range(CT):
                        T.matmul(
                            ps[:, jc * 512:(jc + 1) * 512],
                            lhsT=wT[:, ci_t, co_t * 128:(co_t + 1) * 128],
                            rhs=src_tiles[ci_t][:, jc * 512:(jc + 1) * 512],
                            start=(ci_t == 0), stop=(ci_t == CT - 1))
                if spill is not None:
                    t = outbuf.tile([128, W], F32, name="ob")
                else:
                    t = pme.tile([128, W], F32, name=f"{prefix}{co_t}")
                S.activation(t[:], ps[:], AF.Identity,
                             bias=bias_cols[:, co_t:co_t + 1])
                if spill is not None:
                    nc.sync.dma_start(
                        out=dram[spill].ap()[co_t * 128:(co_t + 1) * 128, :],
                        in_=t[:])
                res.append(t)
            return res

        for s in range(SPC):
            # ---- reload x_g, g ----
            xgs = big_tiles("xg")
            ggs = big_tiles("g")
            for c in range(CT):
                nc.sync.dma_start(
                    out=xgs[c][:],
                    in_=dram["xg_d"].ap()[s, c * 128:(c + 1) * 128, :])
                nc.sync.dma_start(
                    out=ggs[c][:],
                    in_=dram["g_d"].ap()[s, c * 128:(c + 1) * 128, :])

            # ---- projections on raw x_g / g; pq spilled to DRAM ----
            proj_col(xgs, qwT, qbb, "pj", spill="pq_d")
            pk = proj_col(ggs, kwT, kbb, "pk")
            with tc.tile_pool(name="gr", bufs=1) as grp:
                gr = []
                for c in range(CT):
                    t = grp.tile([128, W], F32R, name=f"gr{c}")
                    V.tensor_copy(out=t[:], in_=ggs[c][:])
                    gr.append(t)
                pvT = projT(gr, vwT, vb_bc, "pvT")

            # ---- center in place (needs the AllReduce result) ----
            for c in range(CT):
                V.tensor_scalar(out=xgs[c][:], in0=xgs[c][:],
                                scalar1=m_xg[:], scalar2=None,
                                op0=ALU.subtract, op1=ALU.bypass)
                V.tensor_scalar(out=ggs[c][:], in0=ggs[c][:],
                                scalar1=m_g[:], scalar2=None,
                                op0=ALU.subtract, op1=ALU.bypass)

            Mp = []
            with tc.tile_pool(name="gxT", bufs=1) as gxp:
                # ---- xcT, gcT via PE transposes ----
                xcT, gcT = [], []
                for kt in range(KT):
                    tx = gxp.tile([128, C], F32, name=f"xcT{kt}")
                    tg = gxp.tile([128, C], F32, name=f"gcT{kt}")
                    for ci_t in range(CT):
                        tp = sm_psum.tile([128, 128], F32, name="smp")
                        T.transpose(tp[:],
                                    xgs[ci_t][:, kt * 128:(kt + 1) * 128],
                                    ident[:])
                        V.tensor_copy(
                            out=tx[:, ci_t * 128:(ci_t + 1) * 128],
                            in_=tp[:])
                        tp2 = sm_psum.tile([128, 128], F32, name="smp")
                        T.transpose(tp2[:],
                                    ggs[ci_t][:, kt * 128:(kt + 1) * 128],
                                    ident[:])
                        V.tensor_copy(
                            out=tg[:, ci_t * 128:(ci_t + 1) * 128],
                            in_=tp2[:])
                    xcT.append(tx)
                    gcT.append(tg)

                # ---- MT[c',c] = sum_k gc[c',k] xc[c,k] ----
                MT = []
                for cpt in range(CT):
                    ps = mm_psum.tile([128, W], F32, name="mm_ps")
                    for kt in range(KT):
                        T.matmul(ps[:, 0:C],
                                 lhsT=gcT[kt][:, cpt * 128:(cpt + 1) * 128],
                                 rhs=xcT[kt][:, 0:C],
                                 start=(kt == 0), stop=(kt == KT - 1))
                    t = pme.tile([128, C], F32, name=f"MT{cpt}")
                    V.tensor_copy(out=t[:], in_=ps[:, 0:C])
                    MT.append(t)

                # ---- Mp[c,j] = sum_c' MT[c',c] pk[c',j] ----
                for ct in range(CT):
                    ps = mm_psum.tile([128, W], F32, name="mm_ps")
                    for jc in range(2):
                        for cpt in range(CT):
                            T.matmul(ps[:, jc * 512:(jc + 1) * 512],
                                     lhsT=MT[cpt][:, ct * 128:(ct + 1) * 128],
                                     rhs=pk[cpt][:, jc * 512:(jc + 1) * 512],
                                     start=(cpt == 0), stop=(cpt == CT - 1))
                    t = pme.tile([128, W], F32, name=f"Mp{ct}")
                    V.tensor_copy(out=t[:], in_=ps[:])
                    Mp.append(t)

            # ---- energy -> softmax -> att^T ----
            attT = [attp.tile([128, W], F32R if OUT_F32R else F32,
                              name=f"attT_{kt}")
                    for kt in range(KT)]
            for it in range(KT):
                pqblk = []
                for ct in range(CT):
                    blk = strm.tile([128, 128], F32, name=f"pqb{ct}")
                    nc.sync.dma_start(
                        out=blk[:],
                        in_=dram["pq_d"].ap()[ct * 128:(ct + 1) * 128,
                                              it * 128:(it + 1) * 128])
                    pqblk.append(blk)
                ps = mm_psum.tile([128, W], F32, name="mm_ps")
                for jc in range(2):
                    for ct in range(CT):
                        T.matmul(ps[:, jc * 512:(jc + 1) * 512],
                                 lhsT=pqblk[ct][:],
                                 rhs=Mp[ct][:, jc * 512:(jc + 1) * 512],
                                 start=(ct == 0), stop=(ct == CT - 1))
                rowmax = nrm.tile([128, 1], F32, name="rowmax")
                V.tensor_reduce(out=rowmax[:], in_=ps[:], axis=AX.X,
                                op=ALU.max)
                nb = nrm.tile([128, 1], F32, name="negb")
                V.tensor_tensor(out=nb[:], in0=rowmax[:], in1=negalpha[:],
                                op=ALU.mult)
                e = outbuf.tile([128, W], F32, name="ob")
                rowsum = nrm.tile([128, 1], F32, name="rowsum")
                S.activation(e[:], ps[:], AF.Exp, bias=nb[:], scale=alpha[:],
                             accum_out=rowsum[:])
                rs = nrm.tile([128, 1], F32, name="rs")
                V.reciprocal(out=rs[:], in_=rowsum[:])
                if OUT_F32R:
                    er = outbuf.tile([128, W], F32R, name="obr")
                    V.tensor_scalar_mul(out=er[:], in0=e[:], scalar1=rs[:])
                    e = er
                else:
                    V.tensor_scalar_mul(out=e[:], in0=e[:], scalar1=rs[:])
                if DEBUG_DUMP and s == 0:
                    en = outbuf.tile([128, W], F32, name="ob")
                    V.tensor_copy(out=en[:], in_=ps[:])
                    nc.sync.dma_start(
                        out=dram["dbg_energy"].ap()
                        [it * 128:(it + 1) * 128, :], in_=en[:])
                    nc.sync.dma_start(
                        out=dram["dbg_att"].ap()[it * 128:(it + 1) * 128, :],
                        in_=e[:])
                for kt in range(KT):
                    if OUT_F32R:
                        tp = sm_psum.tile([128, 128], F32R, name="smp")
                        T.transpose(tp[:], e[:, kt * 128:(kt + 1) * 128],
                                    identr[:])
                    else:
                        tp = sm_psum.tile([128, 128], F32, name="smp")
                        T.transpose(tp[:], e[:, kt * 128:(kt + 1) * 128],
                                    ident[:])
                    V.tensor_copy(out=attT[kt][:, it * 128:(it + 1) * 128],
                                  in_=tp[:])

            if DEBUG_DUMP and s == 0:
                for kt_ in range(KT):
                    nc.sync.dma_start(
                        out=dram["dbg_attT"].ap()
                        [kt_ * 128:(kt_ + 1) * 128, :], in_=attT[kt_][:])

            # ---- out[c,j] = sum_k pv[c,k] att[j,k] ----
            for ct in range(CT):
                ps = mm_psum.tile([128, W], F32, name="mm_ps")
                for jc in range(2):
                    for kt in range(KT):
                        T.matmul(ps[:, jc * 512:(jc + 1) * 512],
                                 lhsT=pvT[kt][:, ct * 128:(ct + 1) * 128],
                                 rhs=attT[kt][:, jc * 512:(jc + 1) * 512],
                                 start=(kt == 0), stop=(kt == KT - 1))
                t = outbuf.tile([128, W], F32, name="ob")
                V.tensor_copy(out=t[:], in_=ps[:])
                nc.sync.dma_start(
                    out=dram["y"].ap()[s, ct * 128:(ct + 1) * 128, :],
                    in_=t[:])


def _build():
    nc = bass.Bass("TRN2", target_bir_lowering=False, debug=False,
                   num_devices=N_CORES)
    dram = {}
    dram["x"] = nc.dram_tensor("x", [SPC, C, W], F32, kind="ExternalInput")
    for nm, shp in [("qw", [C, C, 1]), ("kw", [C, C, 1]), ("vw", [C, C, 1]),
                    ("rw1", [C, C, 3]), ("rw2", [C, C, 3])]:
        dram[nm] = nc.dram_tensor(nm, shp, F32, kind="ExternalInput")
    for nm in ["qb", "kb", "vb", "rb1", "rb2"]:
        dram[nm] = nc.dram_tensor(nm, [C], F32, kind="ExternalInput")
    dram["y"] = nc.dram_tensor("y", [SPC, C, W], F32, kind="ExternalOutput")
    dram["xg_d"] = nc.dram_tensor("xg_d", [SPC, C, W], F32)
    dram["pq_d"] = nc.dram_tensor("pq_d", [C, W], F32)
    dram["g_d"] = nc.dram_tensor("g_d", [SPC, C, W], F32)
    if DEBUG_DUMP:
        dram["dbg_energy"] = nc.dram_tensor("dbg_energy", [W, W], F32,
                                            kind="ExternalOutput")
        dram["dbg_att"] = nc.dram_tensor("dbg_att", [W, W], F32,
                                         kind="ExternalOutput")
        dram["dbg_attT"] = nc.dram_tensor("dbg_attT", [W, W], F32,
                                          kind="ExternalOutput")
    dram["cc_in"] = nc.dram_tensor("cc_in", [1, 4], F32)
    dram["cc_out"] = nc.dram_tensor("cc_out", [1, 4], F32,
                                    addr_space="Shared")

    with tile.TileContext(nc) as tc:
        with ExitStack() as ctx:
            _emit(nc, tc, ctx, dram)
    _split_multiwait(nc)
    return nc


_NC_CACHE = {}


def kernel(**inputs):
    if "nc" not in _NC_CACHE:
        _NC_CACHE["nc"] = _build()
    nc = _NC_CACHE["nc"]
    x = np.ascontiguousarray(np.asarray(inputs["x"], dtype=np.float32))
    common = {}
    for nm in ["qw", "kw", "vw", "rw1", "rw2", "qb", "kb", "vb",
               "rb1", "rb2"]:
        common[nm] = np.ascontiguousarray(
            np.asarray(inputs[nm], dtype=np.float32))
    in_maps = []
    for core in range(N_CORES):
        m = dict(common)
        m["x"] = np.ascontiguousarray(x[core * SPC:(core + 1) * SPC])
        in_maps.append(m)
    res = run_bass_kernel_spmd(nc, in_maps, core_ids=list(range(N_CORES)))
    y = np.concatenate([r["y"] for r in res.results], axis=0)
    return y


# revision 24
# speedup vs baseline: 1.4860x; 1.0440x over previous
"""Trainium2 Bass kernel for nn_CCAR_11579231830663 (dense_transformer).

Data-parallel over batch: 16 samples -> 8 NeuronCores x 2 samples. The global
z-score mean/std of x_g and g are the only cross-core terms; each core
all-reduces 4 scalar sums (sum/sumsq of x_g and g) on-device.

Per sample (C=512, W=1024):
  g   = sin(IN(conv3(x, rw1)+rb1)); g = sin(IN(conv3(g, rw2)+rb2))
  x_g = x + g
  qe  = zscore(x_g)^T . pq,  pq = qw@x_g + qb     (z-score over ALL of x_g)
  ke  = zscore(g)^T  . pk,  pk = kw@g + kb
  energy = qe @ ke ; att = softmax(energy); out = (vw@g+vb) @ att^T
The z-score is folded out: center x_g/g by the global means, compute raw
bilinear forms M1 = xc^T@pq (lhsT for energy), M2 = gc^T@pk (rhs), and fold
alpha = 1/(s_xg*s_g) into the exp: softmax(a*E) = exp(a*(E-rowmax))/sum, a>0.
"""
import sys
sys.path.insert(0, '/opt/trn_rl_repo')

import numpy as np
from contextlib import ExitStack

import concourse.bass as bass
import concourse.tile as tile
from concourse import mybir
from concourse.masks import make_identity
from concourse.bass_utils import run_bass_kernel_spmd

F32 = mybir.dt.float32
AF = mybir.ActivationFunctionType
ALU = mybir.AluOpType
AX = mybir.AxisListType

N_CORES = 8
B, C, W = 16, 512, 1024
SPC = B // N_CORES      # samples per core
CT = C // 128           # channel tiles
KT = W // 128           # width 128-tiles
EPS = 1e-5
NTOT = float(B * C * W)

DEBUG_DUMP = False
OUT_F32R = True
F32R = mybir.dt.float32r
MAGIC = 12582912.0       # 1.5*2^23 fp32 round-to-nearest-int magic
TWOPI = float(2 * np.pi)
INV2PI = float(1.0 / (2 * np.pi))

# ---------------------------------------------------------------------------
# walrus workaround: this container's walrus accepts only a limited number of
# sync waits per instruction; Tile can aggregate more (e.g. the tail drain).
# Split excess waits onto same-engine NOPs placed just before the instruction.
_uid = [0]


def _split_multiwait(nc, limit=1):
    for f in nc.m.functions:
        for bb in f.blocks:
            insts = list(bb.instructions)
            out = []
            changed = False
            for inst in insts:
                si = inst.sync_info
                waits = list(si.on_wait) if si is not None and si.on_wait else []
                if len(waits) > limit:
                    changed = True
                    excess, keep = waits[:-limit], waits[-limit:]
                    si.on_wait = keep
                    inst.sync_info = si
                    for i in range(0, len(excess), limit):
                        chunk = excess[i:i + limit]
                        _uid[0] += 1
                        nop = mybir.InstNoOp(
                            name=f"I-waitsplit-{_uid[0]}", ins=[], outs=[])
                        nop.engine = inst.engine
                        nop.sync_info = mybir.SyncInfo(
                            on_wait=chunk, on_update=[])
                        out.append(nop)
                out.append(inst)
            if changed:
                bb.instructions = out


# ---------------------------------------------------------------------------
def _emit(nc, tc, ctx, dram):
    V = nc.vector
    S = nc.scalar
    T = nc.tensor

    # ---------------- outer pools (small / long-lived) --------------------
    singles = ctx.enter_context(tc.tile_pool(name="singles", bufs=1))
    spool = ctx.enter_context(tc.tile_pool(name="spool", bufs=1))
    nrm = ctx.enter_context(tc.tile_pool(name="nrm", bufs=2))
    outbuf = ctx.enter_context(tc.tile_pool(name="outbuf", bufs=2))
    qkv_w = ctx.enter_context(tc.tile_pool(name="qkv_w", bufs=1))
    big = ctx.enter_context(tc.tile_pool(name="big", bufs=1))
    mm_psum = ctx.enter_context(
        tc.tile_pool(name="mm_psum", bufs=3, space="PSUM"))
    sm_psum = ctx.enter_context(
        tc.tile_pool(name="sm_psum", bufs=2, space="PSUM"))

    ident = singles.tile([128, 128], F32, name="ident")
    make_identity(nc, ident[:])
    identr = singles.tile([128, 128], F32R, name="identr")
    V.tensor_copy(out=identr[:], in_=ident[:])

    ones1 = singles.tile([1, 128], F32, name="ones1")
    V.memset(ones1[:], 1.0)
    ones128 = singles.tile([128, 1], F32, name="ones128")
    V.memset(ones128[:], 1.0)

    def load_bias_cols(name):
        t = singles.tile([128, CT], F32, name=f"{name}_cols")
        src = dram[name].ap().rearrange("(t p) -> p t", p=128)
        nc.sync.dma_start(out=t[:], in_=src)
        return t

    rb1b = load_bias_cols("rb1")
    rb2b = load_bias_cols("rb2")
    qbb = load_bias_cols("qb")
    kbb = load_bias_cols("kb")
    vb_bc = singles.tile([128, C], F32, name="vb_bc")
    nc.sync.dma_start(out=vb_bc[:],
                      in_=bass.AP(tensor=dram["vb"], offset=0,
                                  ap=[[0, 128], [1, C]]))

    # columns: 0 sum_xg, 1 sumsq_xg, 2 sum_g, 3 sumsq_g
    stats_block = spool.tile([128, 4], F32, name="stats_block")
    V.memset(stats_block[:], 0.0)

    # x_g and g live in DRAM between the R phase and the per-sample
    # P/M/E pass; inside each phase they occupy tag-shared SBUF tiles.
    def big_tiles(prefix):
        return [big.tile([128, W], F32, name=f"{prefix}_{c}")
                for c in range(CT)]

    # ---------------- helpers ---------------------------------------------
    def rstd_from_var(varcol):
        """[128,1] biased var -> 1/sqrt(var+eps), Newton-refined."""
        veps = nrm.tile([128, 1], F32, name="veps")
        V.tensor_scalar_add(out=veps[:], in0=varcol, scalar1=EPS)
        s0 = nrm.tile([128, 1], F32, name="s0")
        S.activation(s0[:], veps[:], AF.Sqrt)
        y0 = nrm.tile([128, 1], F32, name="y0")
        V.reciprocal(out=y0[:], in_=s0[:])
        t1 = nrm.tile([128, 1], F32, name="nt1")
        V.tensor_tensor(out=t1[:], in0=y0[:], in1=y0[:], op=ALU.mult)
        V.tensor_tensor(out=t1[:], in0=t1[:], in1=veps[:], op=ALU.mult)
        V.tensor_scalar(out=t1[:], in0=t1[:], scalar1=-0.5, scalar2=1.5,
                        op0=ALU.mult, op1=ALU.add)
        y1 = nrm.tile([128, 1], F32, name="ny1")
        V.tensor_tensor(out=y1[:], in0=y0[:], in1=t1[:], op=ALU.mult)
        return y1

    # ======================= R phase: residual block =======================
    with ExitStack() as rctx:
        conv_w = rctx.enter_context(tc.tile_pool(name="conv_w", bufs=1))
        natp = rctx.enter_context(tc.tile_pool(name="wnat", bufs=2))
        padp = rctx.enter_context(tc.tile_pool(name="padp", bufs=8))
        xre = rctx.enter_context(tc.tile_pool(name="xre", bufs=3))
        scr = rctx.enter_context(tc.tile_pool(name="scr", bufs=2))

        # conv weights rw[cout, cin, k] -> rwT[cin_p, k, cin_t, cout_t, :]
        def load_conv_weightT(name):
            wT = conv_w.tile([128, 3, CT, CT, 128], F32, name=f"{name}T")
            for co_t in range(CT):
                nat = natp.tile([128, C * 3], F32, name="wnat")
                nc.sync.dma_start(
                    out=nat[:],
                    in_=dram[name].ap().rearrange("a b c -> a (b c)")
                    [co_t * 128:(co_t + 1) * 128])
                for k in range(3):
                    for ci_t in range(CT):
                        ps = sm_psum.tile([128, 128], F32, name="smp")
                        src = nat[:, ci_t * 384 + k: (ci_t + 1) * 384: 3]
                        T.transpose(ps[:], src, ident[:])
                        V.tensor_copy(out=wT[:, k, ci_t, co_t, :], in_=ps[:])
            return wT

        rw1T = load_conv_weightT("rw1")
        rw2T = load_conv_weightT("rw2")

        # 1x1 weights [cout, cin, 1] -> wT[cin_p, cin_t, cout]
        def load_1x1_weightT(name, dtype=F32):
            wT = qkv_w.tile([128, CT, C], dtype, name=f"{name}T")
            for co_t in range(CT):
                nat = natp.tile([128, C * 3], F32, name="wnat")
                nc.sync.dma_start(
                    out=nat[:, 0:C],
                    in_=dram[name].ap().rearrange("a b c -> a (b c)")
                    [co_t * 128:(co_t + 1) * 128])
                for ci_t in range(CT):
                    ps = sm_psum.tile([128, 128], F32, name="smp")
                    T.transpose(ps[:], nat[:, ci_t * 128:(ci_t + 1) * 128],
                                ident[:])
                    V.tensor_copy(
                        out=wT[:, ci_t, co_t * 128:(co_t + 1) * 128],
                        in_=ps[:])
            return wT

        qwT = load_1x1_weightT("qw")
        kwT = load_1x1_weightT("kw")
        vwT = load_1x1_weightT("vw", dtype=F32R)

        def conv3(dst_cb, src_tiles, wT):
            """3-tap conv from padded [128, W+2] src tiles; dst_cb(co_t, ps)
            gets the accumulated [128, W] PSUM (bias not applied)."""
            for co_t in range(CT):
                ps = mm_psum.tile([128, W], F32, name="mm_ps")
                for jc in range(2):
                    idx = 0
                    for k in range(3):
                        for ci_t in range(CT):
                            T.matmul(ps[:, jc * 512:(jc + 1) * 512],
                                     lhsT=wT[:, k, ci_t, co_t, :],
                                     rhs=src_tiles[ci_t][:, jc * 512 + k:
                                                         jc * 512 + k + 512],
                                     start=(idx == 0), stop=(idx == 11))
                            idx += 1
                dst_cb(co_t, ps)

        def inorm_sin(dst, srcp, bias_col, out_accum=None):
            """dst <- sin(instance_norm(srcp + bias)); srcp: [128, W] PSUM."""
            t = scr.tile([128, W], F32, name="scrA")
            S.activation(t[:], srcp[:], AF.Identity, bias=bias_col)
            st = nrm.tile([128, 2, 6], F32, name="bn_st")
            V.bn_stats(st[:, 0, :], t[:, 0:512])
            V.bn_stats(st[:, 1, :], t[:, 512:1024])
            mv = nrm.tile([128, 2], F32, name="bn_mv")
            V.bn_aggr(mv[:], st[:])
            rstd = rstd_from_var(mv[:, 1:2])
            w = scr.tile([128, W], F32, name="scrB")
            V.tensor_scalar(out=w[:], in0=t[:], scalar1=mv[:, 0:1],
                            scalar2=rstd[:], op0=ALU.subtract, op1=ALU.mult)
            u = scr.tile([128, W], F32, name="scrA")
            V.tensor_scalar(out=u[:], in0=w[:], scalar1=INV2PI, scalar2=MAGIC,
                            op0=ALU.mult, op1=ALU.add)
            V.tensor_scalar(out=u[:], in0=u[:], scalar1=MAGIC, scalar2=None,
                            op0=ALU.subtract, op1=ALU.bypass)
            V.scalar_tensor_tensor(out=u[:], in0=u[:], scalar=-TWOPI,
                                   in1=w[:], op0=ALU.mult, op1=ALU.add)
            S.activation(dst, u[:], AF.Sin, accum_out=out_accum)

        for s in range(SPC):
            xp = [padp.tile([128, W + 2], F32, name="pad") for _ in range(CT)]
            for c in range(CT):
                V.memset(xp[c][:], 0.0)
                nc.sync.dma_start(
                    out=xp[c][:, 1:W + 1],
                    in_=dram["x"].ap()[s, c * 128:(c + 1) * 128, :])

            g1p = [padp.tile([128, W + 2], F32, name="pad") for _ in range(CT)]
            for c in range(CT):
                V.memset(g1p[c][:], 0.0)

            def c1_cb(co_t, ps):
                inorm_sin(g1p[co_t][:, 1:W + 1], ps, rb1b[:, co_t:co_t + 1])

            conv3(c1_cb, xp, rw1T)

            gloc = big_tiles("g")
            gsum = [None] * CT

            def c2_cb(co_t, ps):
                gsum[co_t] = nrm.tile([128, 1], F32, name=f"gsum{co_t}")
                inorm_sin(gloc[co_t][:], ps, rb2b[:, co_t:co_t + 1],
                          out_accum=gsum[co_t][:])

            conv3(c2_cb, g1p, rw2T)

            # x_g = x + g (x re-streamed, in place in the stream tile),
            # sum/sumsq stats, then spill x_g and g to DRAM
            for c in range(CT):
                xt = xre.tile([128, W], F32, name="xre")
                nc.sync.dma_start(
                    out=xt[:],
                    in_=dram["x"].ap()[s, c * 128:(c + 1) * 128, :])
                xs1 = nrm.tile([128, 1], F32, name="xs1")
                V.scalar_tensor_tensor(out=xt[:], in0=xt[:],
                                       scalar=0.0, in1=gloc[c][:],
                                       op0=ALU.add, op1=ALU.add,
                                       accum_out=xs1[:])
                xs2 = nrm.tile([128, 1], F32, name="xs2")
                sq = scr.tile([128, W], F32, name="scrB")
                V.scalar_tensor_tensor(out=sq[:], in0=xt[:], scalar=0.0,
                                       in1=xt[:], op0=ALU.add, op1=ALU.mult,
                                       accum_out=xs2[:])
                gs2 = nrm.tile([128, 1], F32, name="gs2")
                sq2 = scr.tile([128, W], F32, name="scrB")
                V.scalar_tensor_tensor(out=sq2[:], in0=gloc[c][:], scalar=0.0,
                                       in1=gloc[c][:], op0=ALU.add,
                                       op1=ALU.mult, accum_out=gs2[:])
                nc.sync.dma_start(
                    out=dram["xg_d"].ap()[s, c * 128:(c + 1) * 128, :],
                    in_=xt[:])
                nc.sync.dma_start(
                    out=dram["g_d"].ap()[s, c * 128:(c + 1) * 128, :],
                    in_=gloc[c][:])
                V.tensor_tensor(out=stats_block[:, 0:1],
                                in0=stats_block[:, 0:1], in1=xs1[:],
                                op=ALU.add)
                V.tensor_tensor(out=stats_block[:, 1:2],
                                in0=stats_block[:, 1:2], in1=xs2[:],
                                op=ALU.add)
                V.tensor_tensor(out=stats_block[:, 2:3],
                                in0=stats_block[:, 2:3], in1=gsum[c][:],
                                op=ALU.add)
                V.tensor_tensor(out=stats_block[:, 3:4],
                                in0=stats_block[:, 3:4], in1=gs2[:],
                                op=ALU.add)

    # ====================== AllReduce of the 4 sums ========================
    ps4 = sm_psum.tile([128, 128], F32, name="smp")
    T.matmul(ps4[:1, 0:4], lhsT=ones128[:], rhs=stats_block[:],
             start=True, stop=True)
    cc_sb = spool.tile([1, 4], F32, name="cc_sb")
    V.tensor_copy(out=cc_sb[:], in_=ps4[:1, 0:4])
    nc.sync.dma_start(out=dram["cc_in"].ap(), in_=cc_sb[:])
    nc.gpsimd.collective_compute(
        "AllReduce", ALU.add,
        replica_groups=[list(range(N_CORES))],
        ins=[dram["cc_in"].ap()],
        outs=[dram["cc_out"].ap()],
    )
    gstat = spool.tile([128, 4], F32, name="gstat")
    bcast = bass.AP(tensor=dram["cc_out"], offset=0, ap=[[0, 128], [1, 4]])
    nc.sync.dma_start(out=gstat[:], in_=bcast)

    # m = S1/N ; var = (S2 - S1^2/N)/(N-1) ; rs = 1/sqrt(var) (Newton)
    def mean_rs(s1col, s2col, tag):
        m = spool.tile([128, 1], F32, name=f"m_{tag}")
        V.tensor_scalar_mul(out=m[:], in0=s1col, scalar1=1.0 / NTOT)
        t = spool.tile([128, 1], F32, name=f"v_{tag}")
        V.tensor_tensor(out=t[:], in0=s1col, in1=m[:], op=ALU.mult)
        V.tensor_scalar_mul(out=t[:], in0=t[:], scalar1=-1.0)
        V.tensor_tensor(out=t[:], in0=t[:], in1=s2col, op=ALU.add)
        V.tensor_scalar_mul(out=t[:], in0=t[:], scalar1=1.0 / (NTOT - 1.0))
        sq = spool.tile([128, 1], F32, name=f"sq_{tag}")
        S.activation(sq[:], t[:], AF.Sqrt)
        y0 = spool.tile([128, 1], F32, name=f"y0_{tag}")
        V.reciprocal(out=y0[:], in_=sq[:])
        t2 = spool.tile([128, 1], F32, name=f"t2_{tag}")
        V.tensor_tensor(out=t2[:], in0=y0[:], in1=y0[:], op=ALU.mult)
        V.tensor_tensor(out=t2[:], in0=t2[:], in1=t[:], op=ALU.mult)
        V.tensor_scalar(out=t2[:], in0=t2[:], scalar1=-0.5, scalar2=1.5,
                        op0=ALU.mult, op1=ALU.add)
        V.tensor_tensor(out=t2[:], in0=y0[:], in1=t2[:], op=ALU.mult)
        return m, t2

    m_xg, rs_xg = mean_rs(gstat[:, 0:1], gstat[:, 1:2], "xg")
    m_g, rs_g = mean_rs(gstat[:, 2:3], gstat[:, 3:4], "g")
    alpha = spool.tile([128, 1], F32, name="alpha")
    V.tensor_tensor(out=alpha[:], in0=rs_xg[:], in1=rs_g[:], op=ALU.mult)
    negalpha = spool.tile([128, 1], F32, name="negalpha")
    V.tensor_scalar_mul(out=negalpha[:], in0=alpha[:], scalar1=-1.0)

    # =================== P/E phases, one sample at a time ==================
    # Reference algebra: energy = a * pq^T @ (xc @ gc^T) @ pk, a = 1/(sx*sg)
    #   MT[c',c] = sum_k gc[c',k] xc[c,k]      (via gcT, xcT: k-partitioned)
    #   Mp[c,j]  = sum_c' MT[c',c] pk[c',j]
    #   E[i,j]   = sum_c pq[c,i] Mp[c,j]
    with ExitStack() as ectx:
        pme = ectx.enter_context(tc.tile_pool(name="pme", bufs=1))
        attp = ectx.enter_context(tc.tile_pool(name="attp", bufs=1))
        strm = ectx.enter_context(tc.tile_pool(name="strm", bufs=2))

        def projT(src_tiles, wT, b_bc, prefix):
            """proj^T[k, c] = sum_ci src[ci, k] w[c, ci] + b[c] (f32r)"""
            res = []
            for kt in range(KT):
                ps = mm_psum.tile([128, W], F32, name="mm_ps")
                for ci_t in range(CT):
                    T.matmul(ps[:, 0:C],
                             lhsT=src_tiles[ci_t][:, kt * 128:(kt + 1) * 128],
                             rhs=wT[:, ci_t, :],
                             start=(ci_t == 0), stop=(ci_t == CT - 1))
                t = pme.tile([128, C], F32R, name=f"{prefix}{kt}")
                V.scalar_tensor_tensor(out=t[:], in0=ps[:, 0:C], scalar=0.0,
                                       in1=b_bc[:], op0=ALU.add, op1=ALU.add)
                res.append(t)
            return res

        def proj_col(src_tiles, wT, bias_cols, prefix, spill=None):
            """proj[c, j] natural column form; optionally DMA to DRAM
            (spilled projections use rotating outbuf slots)."""
            res = []
            for co_t in range(CT):
                ps = mm_psum.tile([128, W], F32, name="mm_ps")
                for jc in range(2):
                    for ci_t in range(CT):
                        T.matmul(
                            ps[:, jc * 512:(jc + 1) * 512],
                            lhsT=wT[:, ci_t, co_t * 128:(co_t + 1) * 128],
                            rhs=src_tiles[ci_t][:, jc * 512:(jc + 1) * 512],
                            start=(ci_t == 0), stop=(ci_t == CT - 1))
                if spill is not None:
                    t = outbuf.tile([128, W], F32, name="ob")
                else:
                    t = pme.tile([128, W], F32, name=f"{prefix}{co_t}")
                S.activation(t[:], ps[:], AF.Identity,
                             bias=bias_cols[:, co_t:co_t + 1])
                if spill is not None:
                    nc.sync.dma_start(
                        out=dram[spill].ap()[co_t * 128:(co_t + 1) * 128, :],
                        in_=t[:])
                res.append(t)
            return res

        for s in range(SPC):
            # ---- reload x_g, g ----
            xgs = big_tiles("xg")
            ggs = big_tiles("g")
            for c in range(CT):
                nc.sync.dma_start(
                    out=xgs[c][:],
                    in_=dram["xg_d"].ap()[s, c * 128:(c + 1) * 128, :])
                nc.sync.dma_start(
                    out=ggs[c][:],
                    in_=dram["g_d"].ap()[s, c * 128:(c + 1) * 128, :])

            # ---- projections on raw x_g / g; pq spilled to DRAM ----
            proj_col(xgs, qwT, qbb, "pj", spill="pq_d")
            pk = proj_col(ggs, kwT, kbb, "pk")
            with tc.tile_pool(name="gr", bufs=1) as grp:
                gr = []
                for c in range(CT):
                    t = grp.tile([128, W], F32R, name=f"gr{c}")
                    V.tensor_copy(out=t[:], in_=ggs[c][:])
                    gr.append(t)
                pvT = projT(gr, vwT, vb_bc, "pvT")

            # ---- center in place (needs the AllReduce result) ----
            for c in range(CT):
                V.tensor_scalar(out=xgs[c][:], in0=xgs[c][:],
                                scalar1=m_xg[:], scalar2=None,
                                op0=ALU.subtract, op1=ALU.bypass)
                V.tensor_scalar(out=ggs[c][:], in0=ggs[c][:],
                                scalar1=m_g[:], scalar2=None,
                                op0=ALU.subtract, op1=ALU.bypass)

            Mp = []
            with tc.tile_pool(name="gxT", bufs=1) as gxp:
                # ---- xcT, gcT via PE transposes ----
                xcT, gcT = [], []
                for kt in range(KT):
                    tx = gxp.tile([128, C], F32, name=f"xcT{kt}")
                    tg = gxp.tile([128, C], F32, name=f"gcT{kt}")
                    for ci_t in range(CT):
                        tp = sm_psum.tile([128, 128], F32, name="smp")
                        T.transpose(tp[:],
                                    xgs[ci_t][:, kt * 128:(kt + 1) * 128],
                                    ident[:])
                        V.tensor_copy(
                            out=tx[:, ci_t * 128:(ci_t + 1) * 128],
                            in_=tp[:])
                        tp2 = sm_psum.tile([128, 128], F32, name="smp")
                        T.transpose(tp2[:],
                                    ggs[ci_t][:, kt * 128:(kt + 1) * 128],
                                    ident[:])
                        V.tensor_copy(
                            out=tg[:, ci_t * 128:(ci_t + 1) * 128],
                            in_=tp2[:])
                    xcT.append(tx)
                    gcT.append(tg)

                # ---- MT[c',c] = sum_k gc[c',k] xc[c,k] ----
                MT = []
                for cpt in range(CT):
                    ps = mm_psum.tile([128, W], F32, name="mm_ps")
                    for kt in range(KT):
                        T.matmul(ps[:, 0:C],
                                 lhsT=gcT[kt][:, cpt * 128:(cpt + 1) * 128],
                                 rhs=xcT[kt][:, 0:C],
                                 start=(kt == 0), stop=(kt == KT - 1))
                    t = pme.tile([128, C], F32, name=f"MT{cpt}")
                    V.tensor_copy(out=t[:], in_=ps[:, 0:C])
                    MT.append(t)

                # ---- Mp[c,j] = sum_c' MT[c',c] pk[c',j] ----
                for ct in range(CT):
                    ps = mm_psum.tile([128, W], F32, name="mm_ps")
                    for jc in range(2):
                        for cpt in range(CT):
                            T.matmul(ps[:, jc * 512:(jc + 1) * 512],
                                     lhsT=MT[cpt][:, ct * 128:(ct + 1) * 128],
                                     rhs=pk[cpt][:, jc * 512:(jc + 1) * 512],
                                     start=(cpt == 0), stop=(cpt == CT - 1))
                    t = pme.tile([128, W], F32, name=f"Mp{ct}")
                    V.tensor_copy(out=t[:], in_=ps[:])
                    Mp.append(t)

            # ---- energy -> softmax -> att^T ----
            attT = [attp.tile([128, W], F32R if OUT_F32R else F32,
                              name=f"attT_{kt}")
                    for kt in range(KT)]
            for it in range(KT):
                pqblk = []
                for ct in range(CT):
                    blk = strm.tile([128, 128], F32, name=f"pqb{ct}")
                    nc.sync.dma_start(
                        out=blk[:],
                        in_=dram["pq_d"].ap()[ct * 128:(ct + 1) * 128,
                                              it * 128:(it + 1) * 128])
                    pqblk.append(blk)
                ps = mm_psum.tile([128, W], F32, name="mm_ps")
                for jc in range(2):
                    for ct in range(CT):
                        T.matmul(ps[:, jc * 512:(jc + 1) * 512],
                                 lhsT=pqblk[ct][:],
                                 rhs=Mp[ct][:, jc * 512:(jc + 1) * 512],
                                 start=(ct == 0), stop=(ct == CT - 1))
                rowmax = nrm.tile([128, 1], F32, name="rowmax")
                V.tensor_reduce(out=rowmax[:], in_=ps[:], axis=AX.X,
                                op=ALU.max)
                nb = nrm.tile([128, 1], F32, name="negb")
                V.tensor_tensor(out=nb[:], in0=rowmax[:], in1=negalpha[:],
                                op=ALU.mult)
                e = pme.tile([128, W], F32, name="esb")
                rowsum = nrm.tile([128, 1], F32, name="rowsum")
                S.activation(e[:], ps[:], AF.Exp, bias=nb[:], scale=alpha[:],
                             accum_out=rowsum[:])
                rs = nrm.tile([128, 1], F32, name="rs")
                V.reciprocal(out=rs[:], in_=rowsum[:])
                if OUT_F32R:
                    er = outbuf.tile([128, W], F32R, name="obr")
                    V.tensor_scalar_mul(out=er[:], in0=e[:], scalar1=rs[:])
                    e = er
                else:
                    V.tensor_scalar_mul(out=e[:], in0=e[:], scalar1=rs[:])
                if DEBUG_DUMP and s == 0:
                    en = outbuf.tile([128, W], F32, name="ob")
                    V.tensor_copy(out=en[:], in_=ps[:])
                    nc.sync.dma_start(
                        out=dram["dbg_energy"].ap()
                        [it * 128:(it + 1) * 128, :], in_=en[:])
                    nc.sync.dma_start(
                        out=dram["dbg_att"].ap()[it * 128:(it + 1) * 128, :],
                        in_=e[:])
                for kt in range(KT):
                    if OUT_F32R:
                        tp = sm_psum.tile([128, 128], F32R, name="smp")
                        T.transpose(tp[:], e[:, kt * 128:(kt + 1) * 128],
                                    identr[:])
                    else:
                        tp = sm_psum.tile([128, 128], F32, name="smp")
                        T.transpose(tp[:], e[:, kt * 128:(kt + 1) * 128],
                                    ident[:])
                    V.tensor_copy(out=attT[kt][:, it * 128:(it + 1) * 128],
                                  in_=tp[:])

            if DEBUG_DUMP and s == 0:
                for kt_ in range(KT):
                    nc.sync.dma_start(
                        out=dram["dbg_attT"].ap()
                        [kt_ * 128:(kt_ + 1) * 128, :], in_=attT[kt_][:])

            # ---- out[c,j] = sum_k pv[c,k] att[j,k] ----
            for ct in range(CT):
                ps = mm_psum.tile([128, W], F32, name="mm_ps")
                for jc in range(2):
                    for kt in range(KT):
                        T.matmul(ps[:, jc * 512:(jc + 1) * 512],
                                 lhsT=pvT[kt][:, ct * 128:(ct + 1) * 128],
                                 rhs=attT[kt][:, jc * 512:(jc + 1) * 512],
                                 start=(kt == 0), stop=(kt == KT - 1))
                t = outbuf.tile([128, W], F32, name="ob")
                V.tensor_copy(out=t[:], in_=ps[:])
                nc.sync.dma_start(
                    out=dram["y"].ap()[s, ct * 128:(ct + 1) * 128, :],
                    in_=t[:])


def _build():
    nc = bass.Bass("TRN2", target_bir_lowering=False, debug=False,
                   num_devices=N_CORES)
    dram = {}
    dram["x"] = nc.dram_tensor("x", [SPC, C, W], F32, kind="ExternalInput")
    for nm, shp in [("qw", [C, C, 1]), ("kw", [C, C, 1]), ("vw", [C, C, 1]),
                    ("rw1", [C, C, 3]), ("rw2", [C, C, 3])]:
        dram[nm] = nc.dram_tensor(nm, shp, F32, kind="ExternalInput")
    for nm in ["qb", "kb", "vb", "rb1", "rb2"]:
        dram[nm] = nc.dram_tensor(nm, [C], F32, kind="ExternalInput")
    dram["y"] = nc.dram_tensor("y", [SPC, C, W], F32, kind="ExternalOutput")
    dram["xg_d"] = nc.dram_tensor("xg_d", [SPC, C, W], F32)
    dram["pq_d"] = nc.dram_tensor("pq_d", [C, W], F32)
    dram["g_d"] = nc.dram_tensor("g_d", [SPC, C, W], F32)
    if DEBUG_DUMP:
        dram["dbg_energy"] = nc.dram_tensor("dbg_energy", [W, W], F32,
                                            kind="ExternalOutput")
        dram["dbg_att"] = nc.dram_tensor("dbg_att", [W, W], F32,
                                         kind="ExternalOutput")
        dram["dbg_attT"] = nc.dram_tensor("dbg_attT", [W, W], F32,
                                          kind="ExternalOutput")
    dram["cc_in"] = nc.dram_tensor("cc_in", [1, 4], F32)
    dram["cc_out"] = nc.dram_tensor("cc_out", [1, 4], F32,
                                    addr_space="Shared")

    with tile.TileContext(nc) as tc:
        with ExitStack() as ctx:
            _emit(nc, tc, ctx, dram)
    _split_multiwait(nc)
    return nc


_NC_CACHE = {}


def kernel(**inputs):
    if "nc" not in _NC_CACHE:
        _NC_CACHE["nc"] = _build()
    nc = _NC_CACHE["nc"]
    x = np.ascontiguousarray(np.asarray(inputs["x"], dtype=np.float32))
    common = {}
    for nm in ["qw", "kw", "vw", "rw1", "rw2", "qb", "kb", "vb",
               "rb1", "rb2"]:
        common[nm] = np.ascontiguousarray(
            np.asarray(inputs[nm], dtype=np.float32))
    in_maps = []
    for core in range(N_CORES):
        m = dict(common)
        m["x"] = np.ascontiguousarray(x[core * SPC:(core + 1) * SPC])
        in_maps.append(m)
    res = run_bass_kernel_spmd(nc, in_maps, core_ids=list(range(N_CORES)))
    y = np.concatenate([r["y"] for r in res.results], axis=0)
    return y
